# revision 1
# baseline (speedup 1.0000x reference)
"""Trainium2 Bass kernel for nn_AE_29171417875247 (k-sparse autoencoder with
top-k masking).

  h1 = sigmoid(x @ enc_W0 + enc_b0)        [B, 2048]
  h2 = sigmoid(h1 @ enc_W1 + enc_b1)       [B, 1024]
  h2 = keep top-51 per row, zero rest      (k = 1024 * 0.05)
  d  = sigmoid(h2 @ dec_W1 + dec_b1)       [B, 2048]
  out = d @ dec_W0 + dec_b0                [B, 4096]

Data-parallel across 8 NeuronCores: each core owns 1024 rows of the batch
and the full (replicated) weights. Matmuls run in float32r (fp32 with
11-bit mantissa, full PE rate); inputs are pre-rounded to f32r on the host
(round-to-nearest-even, keep 11 mantissa bits) which matches the hardware
cast bit-exactly. Top-k selection runs on exact f32 sigmoid outputs via the
DVE max8 + match_replace instructions (7 rounds of 8, last round keeps 3),
identical semantics to jax.lax.top_k for distinct values.

Per-core dataflow (rows on the moving/free side so N=512 per matmul; all
activations stay in SBUF, no DRAM staging):
  MM1: h1T[m,rows] += W0[k,m].T @ xT[k,rows]  (W0 quarter-slabs streamed,
       xT resident) -> sigmoid+b0 -> h1T resident in SBUF (f32r, 8MB)
  MM2: h2[rows,n]  += h1T[kk,rows].T @ W1[kk,n] (+ rank-1 bias matmul,
       W1 fully resident) -> sigmoid -> h2 [rows on partitions]
  topk: 7x(max8 + match_replace) -> zapped; hmask = h2 - zapped  (DVE,
       overlapped with the next row-tile's MM2 groups)
  PE-transpose hmask -> hmaskT (f32r), emission delayed 2 row-tiles so the
       in-order PE does not stall on the DVE chain
  MM3: dT[m,rows] += dW1[kk,m].T @ hmaskT[kk,rows] -> sigmoid+db1 -> dT (f32r)
  MM4: outT[m,rows] += dW0[kk,m].T @ dT[kk,rows] -> +db0 -> DRAM
Host transposes outT shards back and concatenates.
"""
import sys
sys.path.insert(0, '/opt/trn_rl_repo')
import numpy as np

B, D, H1, H2 = 8192, 4096, 2048, 1024
NCORES = 8
BC = B // NCORES          # rows per core = 1024
K_TOP = 51                # int(H2 * 0.05)
KD = D // 128             # 32 k-chunks for MM1
KH1 = H1 // 128           # 16
KH2 = H2 // 128           # 8
M1 = H1 // 128            # 16 h1 tiles
M3 = H1 // 128            # 16 dT tiles
M4 = D // 128             # 32 out tiles
NR = BC // 512            # 2 row halves of 512


def _round_f32r(x: np.ndarray) -> np.ndarray:
    """Round f32 -> f32r (keep 11 mantissa bits, round-to-nearest-even).
    Bit-exact match to the hardware's f32->f32r cast (verified on silicon)."""
    b = np.ascontiguousarray(x, dtype=np.float32).view(np.uint32).astype(np.uint64)
    shift = 23 - 11
    add = (1 << (shift - 1)) - 1 + ((b >> shift) & 1)
    out = ((b + add) >> shift) << shift
    return out.astype(np.uint32).view(np.float32)


def _build(loop_k: int = 1, stages: str = 'ABCD'):
    import contextlib
    import concourse.bacc as bacc
    import concourse.mybir as mybir
    import concourse.tile as tile

    f32 = mybir.dt.float32
    f32r = mybir.dt.float32r
    SIG = mybir.ActivationFunctionType.Sigmoid

    nc = bacc.Bacc("TRN2", target_bir_lowering=False, debug=False)
    XTR = nc.dram_tensor("XTR", (128, KD * BC), f32r, kind="ExternalInput").ap()
    W0R = nc.dram_tensor("W0R", (M1, 128, KD * 128), f32r, kind="ExternalInput").ap()
    W1R = nc.dram_tensor("W1R", (2, 128, KH1 * 512), f32r, kind="ExternalInput").ap()
    DW1R = nc.dram_tensor("DW1R", (M3, 128, KH2 * 128), f32r, kind="ExternalInput").ap()
    DW0R = nc.dram_tensor("DW0R", (M4, 128, KH1 * 128), f32r, kind="ExternalInput").ap()
    B1R = nc.dram_tensor("B1R", (1, H2), f32r, kind="ExternalInput").ap()
    B0R = nc.dram_tensor("B0R", (128, M1), f32, kind="ExternalInput").ap()
    DB1R = nc.dram_tensor("DB1R", (128, M3), f32, kind="ExternalInput").ap()
    DB0R = nc.dram_tensor("DB0R", (128, M4), f32, kind="ExternalInput").ap()
    IDENT = nc.dram_tensor("IDENT", (128, 128), f32, kind="ExternalInput").ap()
    ONESR = nc.dram_tensor("ONESR", (1, 128), f32r, kind="ExternalInput").ap()
    OUTT = nc.dram_tensor("OUTT", (M4, 128, BC), f32, kind="ExternalOutput").ap()

    with tile.TileContext(nc) as tc:
        loop_cm = tc.For_i(0, loop_k, 1) if loop_k > 1 else contextlib.nullcontext()
        with loop_cm, \
             tc.tile_pool(name="biasp", bufs=1) as biasp, \
             tc.tile_pool(name="psum", bufs=6, space="PSUM") as psp:
            # merged per-partition biases: [b0 | db1 | db0]
            biases = biasp.tile([128, M1 + M3 + M4], f32)
            nc.sync.dma_start(biases[:, 0:M1], B0R)
            nc.sync.dma_start(biases[:, M1:M1 + M3], DB1R)
            nc.sync.dma_start(biases[:, M1 + M3:], DB0R)
            b0t = biases[:, 0:M1]
            db1t = biases[:, M1:M1 + M3]
            db0t = biases[:, M1 + M3:M1 + M3 + M4]

            # h1T lives in SBUF across stages A and B (no DRAM staging):
            # [128 H1-partitions x (m-tile, 1024 rows)]
            with tc.tile_pool(name="h1T", bufs=1) as h1Tp:
                h1T = h1Tp.tile([128, M1 * BC], f32r)

                # ---- Stage A: h1T[m] = sigmoid(W0[m].T @ xT + b0[m]) ----
                with tc.tile_pool(name="xt", bufs=1) as xtp, \
                     tc.tile_pool(name="w0", bufs=3) as w0p:
                    xt = xtp.tile([128, KD * BC], f32r)
                    QW = KD * 128 // 4          # W0 slab quarter (8 k-chunks)
                    # m=0's W0 quarters go first on the sync ring, then xt
                    # splits across both rings so MM1 starts ~35us in
                    w0q0 = [None] * 4
                    for q in range(4):
                        w0q0[q] = w0p.tile([128, QW], f32r, tag="w0q",
                                           name=f"w0q0{q}")
                        nc.sync.dma_start(w0q0[q][:],
                                          W0R[0, :, q * QW:(q + 1) * QW])
                    step = KD * BC // 8
                    for i in range(8):
                        nc.scalar.dma_start(xt[:, i * step:(i + 1) * step],
                                            XTR[:, i * step:(i + 1) * step])
                    for m in range(M1):
                        if m == 0:
                            w0q = w0q0
                        else:
                            w0q = [None] * 4
                            for q in range(4):
                                w0q[q] = w0p.tile([128, QW], f32r, tag="w0q",
                                                  name=f"w0q{q}")
                                # split the W0 stream across both HWDGE rings
                                # (scalar ring is idle once xt has landed)
                                eng = nc.sync if q < 2 else nc.scalar
                                eng.dma_start(w0q[q][:],
                                              W0R[m, :, q * QW:(q + 1) * QW])
                        for n in range(NR):
                            ps = psp.tile([128, 512], f32)
                            for k in range(KD):
                                wq = w0q[k // 8]
                                off = (k % 8) * 128
                                nc.tensor.matmul(
                                    ps[:], wq[:, off:off + 128],
                                    xt[:, k * BC + n * 512:
                                       k * BC + n * 512 + 512],
                                    start=(k == 0), stop=(k == KD - 1))
                            nc.scalar.activation(
                                h1T[:, m * BC + n * 512: m * BC + n * 512 + 512],
                                ps[:], SIG, bias=b0t[:, m:m + 1])

                # ---- Stage B: MM2 + top-k + transpose ----
                with tc.tile_pool(name="hmT", bufs=1) as hmTp, \
                     tc.tile_pool(name="dw1", bufs=2) as dw1p:
                    hmT = [hmTp.tile([128, BC], f32r, tag=f"hmT{k}",
                                     name=f"hmT{k}")
                           for k in range(KH2)]

                    with tc.tile_pool(name="w1", bufs=1) as w1p, \
                         tc.tile_pool(name="h2", bufs=2) as h2p, \
                         tc.tile_pool(name="tk", bufs=1) as tkp, \
                         tc.tile_pool(name="hmp", bufs=3) as hmp, \
                         tc.tile_pool(name="mx8", bufs=1) as mxp, \
                         tc.tile_pool(name="cst2", bufs=1) as cst2p, \
                         tc.tile_pool(name="tps", bufs=2, space="PSUM") as tpsp:
                        ident = cst2p.tile([128, 128], f32)
                        nc.sync.dma_start(ident[:], IDENT)
                        b1t = cst2p.tile([1, H2], f32r)
                        nc.sync.dma_start(b1t[:], B1R)
                        ones1 = cst2p.tile([1, 128], f32r)
                        nc.sync.dma_start(ones1[:], ONESR)
                        # both W1 halves resident; 4 quarter-DMAs so the
                        # first MM2 group starts early
                        w1h = w1p.tile([128, 2 * KH1 * 512], f32r, tag="w1h")
                        if 'B' in stages:
                            for q in range(4):
                                n_, fr = q // 2, q % 2
                                HF = KH1 * 256
                                eng = nc.sync if q % 2 == 0 else nc.scalar
                                eng.dma_start(
                                    w1h[:, q * HF:(q + 1) * HF],
                                    W1R[n_, :, fr * HF:(fr + 1) * HF])
                        pending = []

                        def flush_transposes():
                            rr, hm = pending.pop(0)
                            for kk in range(KH2):
                                pst = tpsp.tile([128, 128], f32, name="pst")
                                nc.tensor.transpose(
                                    pst[:], hm[:, kk * 128:(kk + 1) * 128],
                                    ident[:])
                                nc.scalar.copy(
                                    hmT[kk][:, rr * 128:(rr + 1) * 128],
                                    pst[:])

                        for r in range(NR * 4 if 'B' in stages else 0):
                            h2r = h2p.tile([128, H2], f32, tag="h2")
                            for n in range(2):
                                ps = psp.tile([128, 512], f32)
                                for kk in range(KH1):
                                    nc.tensor.matmul(
                                        ps[:],
                                        h1T[:, kk * BC + r * 128:
                                            kk * BC + r * 128 + 128],
                                        w1h[:, (n * KH1 + kk) * 512:
                                            (n * KH1 + kk) * 512 + 512],
                                        start=(kk == 0), stop=False)
                                nc.tensor.matmul(ps[:], ones1[:],
                                                 b1t[:, n * 512:(n + 1) * 512],
                                                 start=False, stop=True)
                                nc.scalar.activation(
                                    h2r[:, n * 512:(n + 1) * 512], ps[:], SIG)
                            # top-51 mask for row tile r
                            zap = tkp.tile([128, H2], f32, tag="zap")
                            cur = h2r
                            for it in range(7):
                                mx = mxp.tile([128, 8], f32, tag="mx")
                                nc.vector.max(mx[:], cur[:])
                                if it == 6:
                                    nc.vector.memset(mx[:, 3:8], 0.0)
                                nc.vector.match_replace(
                                    out=zap[:], in_to_replace=mx[:],
                                    in_values=cur[:], imm_value=0.0)
                                cur = zap
                            hmask = hmp.tile([128, H2], f32, tag="hmask")
                            nc.vector.tensor_sub(hmask[:], h2r[:], zap[:])
                            pending.append((r, hmask))
                            if len(pending) > 2:
                                flush_transposes()
                        while pending:
                            flush_transposes()

                    # ---- Stage C: dT = sigmoid(dW1.T @ hmaskT + db1) ----
                    with tc.tile_pool(name="dT", bufs=1) as dTp:
                        dT = [dTp.tile([128, BC], f32r, tag=f"dT{m}",
                                       name=f"dT{m}")
                              for m in range(M3)]
                        for m in range(M3 if 'C' in stages else 0):
                            dw1s = dw1p.tile([128, KH2 * 128], f32r)
                            nc.sync.dma_start(dw1s[:], DW1R[m])
                            for n2 in range(NR):
                                ps = psp.tile([128, 512], f32)
                                for kk in range(KH2):
                                    nc.tensor.matmul(
                                        ps[:], dw1s[:, kk * 128:(kk + 1) * 128],
                                        hmT[kk][:, n2 * 512:(n2 + 1) * 512],
                                        start=(kk == 0), stop=(kk == KH2 - 1))
                                nc.scalar.activation(
                                    dT[m][:, n2 * 512:(n2 + 1) * 512], ps[:],
                                    SIG, bias=db1t[:, m:m + 1])

                        # ---- Stage D: outT = dW0.T @ dT + db0 -> DRAM ----
                        with tc.tile_pool(name="dw0", bufs=2) as dw0p, \
                             tc.tile_pool(name="outp", bufs=3) as outp:
                            for m in range(M4 if 'D' in stages else 0):
                                dw0s = dw0p.tile([128, KH1 * 128], f32r)
                                nc.sync.dma_start(dw0s[:], DW0R[m])
                                om = outp.tile([128, BC], f32)
                                for n2 in range(NR):
                                    ps = psp.tile([128, 512], f32)
                                    for kk in range(KH1):
                                        nc.tensor.matmul(
                                            ps[:],
                                            dw0s[:, kk * 128:(kk + 1) * 128],
                                            dT[kk][:, n2 * 512:(n2 + 1) * 512],
                                            start=(kk == 0),
                                            stop=(kk == KH1 - 1))
                                    nc.vector.tensor_scalar_add(
                                        om[:, n2 * 512:(n2 + 1) * 512], ps[:],
                                        db0t[:, m:m + 1])
                                nc.scalar.dma_start(OUTT[m], om[:])
    nc.compile()
    return nc


_NC_CACHE = None


def _get_nc():
    global _NC_CACHE
    if _NC_CACHE is None:
        _NC_CACHE = _build()
    return _NC_CACHE


def _build_looped(loop_k: int):
    return _build(loop_k)


def make_in_maps(x, enc_W0, enc_b0, enc_W1, enc_b1, dec_W1, dec_b1, dec_W0,
                 dec_b0):
    w0r = _round_f32r(enc_W0).reshape(KD, 128, M1, 128) \
        .transpose(2, 1, 0, 3).reshape(M1, 128, KD * 128)
    w1r = _round_f32r(enc_W1).reshape(KH1, 128, 2, 512) \
        .transpose(2, 1, 0, 3).reshape(2, 128, KH1 * 512)
    dw1r = _round_f32r(dec_W1).reshape(KH2, 128, M3, 128) \
        .transpose(2, 1, 0, 3).reshape(M3, 128, KH2 * 128)
    dw0r = _round_f32r(dec_W0).reshape(KH1, 128, M4, 128) \
        .transpose(2, 1, 0, 3).reshape(M4, 128, KH1 * 128)
    b1r = _round_f32r(enc_b1).reshape(1, H2)
    b0r = np.ascontiguousarray(enc_b0.reshape(M1, 128).T)
    db1r = np.ascontiguousarray(dec_b1.reshape(M3, 128).T)
    db0r = np.ascontiguousarray(dec_b0.reshape(M4, 128).T)
    ident = np.eye(128, dtype=np.float32)
    shared = dict(W0R=np.ascontiguousarray(w0r), W1R=np.ascontiguousarray(w1r),
                  DW1R=np.ascontiguousarray(dw1r),
                  DW0R=np.ascontiguousarray(dw0r), B1R=b1r, B0R=b0r,
                  DB1R=db1r, DB0R=db0r, IDENT=ident,
                  ONESR=np.ones((1, 128), dtype=np.float32))
    xr = _round_f32r(x)
    in_maps = []
    for c in range(NCORES):
        shard = xr[c * BC:(c + 1) * BC]          # [BC, D]
        xt = np.ascontiguousarray(
            shard.T.reshape(KD, 128, BC).transpose(1, 0, 2)
        ).reshape(128, KD * BC)
        in_maps.append(dict(shared, XTR=xt))
    return in_maps


def kernel(**inputs) -> np.ndarray:
    from concourse import bass_utils
    nc = _get_nc()
    in_maps = make_in_maps(**inputs)
    res = bass_utils.run_bass_kernel_spmd(nc, in_maps,
                                          core_ids=list(range(NCORES)))
    outs = []
    for c in range(NCORES):
        ot = res.results[c]["OUTT"]              # [M4, 128, BC]
        outs.append(ot.reshape(D, BC).T)         # [BC, D]
    return np.ascontiguousarray(np.concatenate(outs, axis=0), dtype=np.float32)



# revision 3
# speedup vs baseline: 1.2058x; 1.2058x over previous
"""Trainium2 Bass kernel for nn_AE_29171417875247 (k-sparse autoencoder with
top-k masking).

  h1 = sigmoid(x @ enc_W0 + enc_b0)        [B, 2048]
  h2 = sigmoid(h1 @ enc_W1 + enc_b1)       [B, 1024]
  h2 = keep top-51 per row, zero rest      (k = 1024 * 0.05)
  d  = sigmoid(h2 @ dec_W1 + dec_b1)       [B, 2048]
  out = d @ dec_W0 + dec_b0                [B, 4096]

Data-parallel across 8 NeuronCores: each core owns 1024 rows of the batch
and the full (replicated) weights. All matmul operands are bf16 (cast on
host, round-to-nearest-even); PSUM accumulation is fp32, and the sigmoid
outputs feeding top-k stay fp32 so the top-51 selection is (near-)exact.
Host-emulated end-to-end bf16 error vs the f32 reference: 4.6e-3.

Per-core pipeline (order chosen so the serial DVE top-k chains hide under
PE-heavy phases; stage A is split 640/384 rows so both top-k batches get a
long PE window):
  A0: h1T rows 0-639   (W0 streamed, xt resident)
  B0-4: MM2+sigmoid rows 0-639 -> 5 DVE top-k chains run during A1
  A1: h1T rows 640-1023 (W0 re-streamed, ~82us of PE to cover the DVE)
  T0-4: PE-transpose hmask tiles 0-4
  B5-7: MM2 rows 640-1023 -> 3 more DVE chains, hidden under C0
  C0: dT[:, 0:512]   = sigmoid(dW1.T @ hmaskT[:, 0:512])
  T5-7, C1: remaining transposes + dT[:, 512:1024]
  D:  outT = dW0.T @ dT + db0 -> DRAM (dw0 streamed once)
"""
import sys
sys.path.insert(0, '/opt/trn_rl_repo')
import numpy as np
import ml_dtypes

BF = ml_dtypes.bfloat16

B, D, H1, H2 = 8192, 4096, 2048, 1024
NCORES = 8
BC = B // NCORES          # rows per core = 1024
K_TOP = 51                # int(H2 * 0.05)
KD = D // 128             # 32 k-chunks for MM1
KH1 = H1 // 128           # 16
KH2 = H2 // 128           # 8
M1 = H1 // 128            # 16 h1 tiles
M3 = H1 // 128            # 16 dT tiles
M4 = D // 128             # 32 out tiles
ROWS_A0 = 640             # stage-A row split: 5 top-k tiles then 3
ROWS_A1 = BC - ROWS_A0    # 384


def _build(loop_k: int = 1, stages: str = 'ABCD'):
    import contextlib
    import concourse.bacc as bacc
    import concourse.mybir as mybir
    import concourse.tile as tile

    f32 = mybir.dt.float32
    bf16 = mybir.dt.bfloat16
    SIG = mybir.ActivationFunctionType.Sigmoid

    nc = bacc.Bacc("TRN2", target_bir_lowering=False, debug=False)
    XTR = nc.dram_tensor("XTR", (128, KD * BC), bf16, kind="ExternalInput").ap()
    W0R = nc.dram_tensor("W0R", (M1, 128, KD * 128), bf16,
                         kind="ExternalInput").ap()
    W1R = nc.dram_tensor("W1R", (2, 128, KH1 * 512), bf16,
                         kind="ExternalInput").ap()
    DW1R = nc.dram_tensor("DW1R", (M3, 128, KH2 * 128), bf16,
                          kind="ExternalInput").ap()
    DW0R = nc.dram_tensor("DW0R", (M4, 128, KH1 * 128), bf16,
                          kind="ExternalInput").ap()
    B1R = nc.dram_tensor("B1R", (1, H2), bf16, kind="ExternalInput").ap()
    B0R = nc.dram_tensor("B0R", (128, M1), f32, kind="ExternalInput").ap()
    DB1R = nc.dram_tensor("DB1R", (128, M3), f32, kind="ExternalInput").ap()
    DB0R = nc.dram_tensor("DB0R", (128, M4), f32, kind="ExternalInput").ap()
    IDENT = nc.dram_tensor("IDENT", (128, 128), bf16, kind="ExternalInput").ap()
    ONESR = nc.dram_tensor("ONESR", (1, 128), bf16, kind="ExternalInput").ap()
    OUTT = nc.dram_tensor("OUTT", (M4, 128, BC), f32, kind="ExternalOutput").ap()

    with tile.TileContext(nc) as tc:
        loop_cm = tc.For_i(0, loop_k, 1) if loop_k > 1 else contextlib.nullcontext()
        with loop_cm, \
             tc.tile_pool(name="biasp", bufs=1) as biasp, \
             tc.tile_pool(name="cstp", bufs=1) as cstp, \
             tc.tile_pool(name="psum", bufs=6, space="PSUM") as psp, \
             tc.tile_pool(name="tps", bufs=2, space="PSUM") as tpsp:
            # merged per-partition biases: [b0 | db1 | db0] (fp32, act bias)
            biases = biasp.tile([128, M1 + M3 + M4], f32)
            nc.sync.dma_start(biases[:, 0:M1], B0R)
            nc.sync.dma_start(biases[:, M1:M1 + M3], DB1R)
            nc.sync.dma_start(biases[:, M1 + M3:], DB0R)
            b0t = biases[:, 0:M1]
            db1t = biases[:, M1:M1 + M3]
            db0t = biases[:, M1 + M3:M1 + M3 + M4]

            with tc.tile_pool(name="h1T", bufs=1) as h1Tp, \
                 tc.tile_pool(name="w1", bufs=1) as w1p, \
                 tc.tile_pool(name="hmT", bufs=1) as hmTp, \
                 tc.tile_pool(name="h2", bufs=3) as h2p, \
                 tc.tile_pool(name="tk", bufs=2) as tkp, \
                 tc.tile_pool(name="hmp", bufs=5) as hmp, \
                 tc.tile_pool(name="mx8", bufs=2) as mxp:
                # h1T: [128 H1-part x (m-tile, 1024 rows)] bf16, SBUF-resident
                h1T = h1Tp.tile([128, M1 * BC], bf16)
                w1h = w1p.tile([128, 2 * KH1 * 512], bf16)
                # hmaskT, kk-major: [128 H2-part x (kk, 1024 rows)]
                hmTt = hmTp.tile([128, KH2 * BC], bf16)
                ident = cstp.tile([128, 128], bf16)
                b1t = cstp.tile([1, H2], bf16)
                ones1 = cstp.tile([1, 128], bf16)

                def mm2_topk(r):
                    """MM2 + sigmoid + top-51 for row tile r (128 rows).
                    PE: 2x(16 mm + rank-1 bias mm); DVE: 7x(max8+match
                    replace); Pool: hmask = h2 - zap (cast to bf16)."""
                    h2r = h2p.tile([128, H2], f32, tag="h2")
                    for n in range(2):
                        ps = psp.tile([128, 512], f32)
                        for kk in range(KH1):
                            nc.tensor.matmul(
                                ps[:],
                                h1T[:, kk * BC + r * 128:
                                    kk * BC + r * 128 + 128],
                                w1h[:, (n * KH1 + kk) * 512:
                                    (n * KH1 + kk) * 512 + 512],
                                start=(kk == 0), stop=False)
                        nc.tensor.matmul(ps[:], ones1[:],
                                         b1t[:, n * 512:(n + 1) * 512],
                                         start=False, stop=True)
                        nc.scalar.activation(h2r[:, n * 512:(n + 1) * 512],
                                             ps[:], SIG)
                    zap = tkp.tile([128, H2], f32, tag="zap")
                    cur = h2r
                    for it in range(7):
                        mx = mxp.tile([128, 8], f32, tag="mx")
                        nc.vector.max(mx[:], cur[:])
                        if it == 6:
                            nc.vector.memset(mx[:, 3:8], 0.0)
                        nc.vector.match_replace(
                            out=zap[:], in_to_replace=mx[:],
                            in_values=cur[:], imm_value=0.0)
                        cur = zap
                    hmask = hmp.tile([128, H2], bf16, tag="hmask")
                    nc.gpsimd.tensor_sub(hmask[:], h2r[:], zap[:])
                    return hmask

                def trans(r, hm):
                    for kk in range(KH2):
                        pst = tpsp.tile([128, 128], bf16, name="pst")
                        nc.tensor.transpose(
                            pst[:], hm[:, kk * 128:(kk + 1) * 128], ident[:])
                        nc.scalar.copy(hmT[kk][:, r * 128:(r + 1) * 128],
                                       pst[:])

                hmasks = {}
                with tc.tile_pool(name="xt", bufs=1) as xtp, \
                     tc.tile_pool(name="w0", bufs=3) as w0p:
                    xt = xtp.tile([128, KD * BC], bf16)
                    # rows 0..ROWS_A0 of every k-chunk first (A0's working set)
                    for k in range(KD):
                        nc.scalar.dma_start(xt[:, k * BC:k * BC + ROWS_A0],
                                            XTR[:, k * BC:k * BC + ROWS_A0])
                    nc.scalar.dma_start(ident[:], IDENT)
                    nc.scalar.dma_start(b1t[:], B1R)
                    nc.scalar.dma_start(ones1[:], ONESR)

                    def stageA(row_off, nrows, half):
                        for m in range(M1):
                            w0s = w0p.tile([128, KD * 128], bf16, tag="w0s",
                                           name=f"w0s{half}_{m}")
                            nc.sync.dma_start(w0s[:], W0R[m])
                            o = 0
                            while o < nrows:
                                w = min(512, nrows - o)
                                ps = psp.tile([128, 512], f32)
                                for k in range(KD):
                                    c = k * BC + row_off + o
                                    nc.tensor.matmul(
                                        ps[:, 0:w], w0s[:, k * 128:(k + 1) * 128],
                                        xt[:, c:c + w],
                                        start=(k == 0), stop=(k == KD - 1))
                                nc.scalar.activation(
                                    h1T[:, m * BC + row_off + o:
                                        m * BC + row_off + o + w],
                                    ps[:, 0:w], SIG, bias=b0t[:, m:m + 1])
                                o += w

                    if 'A' in stages:
                        stageA(0, ROWS_A0, 0)
                    # prefetch: rest of xt, W1 halves (scalar ring has slack)
                    for k in range(KD):
                        nc.scalar.dma_start(
                            xt[:, k * BC + ROWS_A0:k * BC + BC],
                            XTR[:, k * BC + ROWS_A0:k * BC + BC])
                    HF = KH1 * 256
                    for q in range(4):
                        n_, fr = q // 2, q % 2
                        nc.scalar.dma_start(w1h[:, q * HF:(q + 1) * HF],
                                            W1R[n_, :, fr * HF:(fr + 1) * HF])
                    if 'B' in stages:
                        for r in range(5):
                            hmasks[r] = mm2_topk(r)
                    if 'A' in stages:
                        stageA(ROWS_A0, ROWS_A1, 1)

                # xt + W0 pools closed: their SBUF is reused below
                with tc.tile_pool(name="dw1", bufs=1) as dw1p, \
                     tc.tile_pool(name="dT", bufs=1) as dTp, \
                     tc.tile_pool(name="dw0", bufs=3) as dw0p, \
                     tc.tile_pool(name="outp", bufs=4) as outp:
                    dw1 = dw1p.tile([128, M3 * KH2 * 128], bf16)
                    for m in range(M3):
                        nc.sync.dma_start(
                            dw1[:, m * KH2 * 128:(m + 1) * KH2 * 128],
                            DW1R[m])
                    dT = [dTp.tile([128, BC], bf16, tag=f"dT{m}",
                                   name=f"dT{m}") for m in range(M3)]

                    if 'B' in stages:
                        for r in range(5):
                            trans(r, hmasks.pop(r))
                        for r in range(5, 8):
                            hmasks[r] = mm2_topk(r)

                    def stageC(n2):
                        for m in range(M3):
                            ps = psp.tile([128, 512], f32)
                            for kk in range(KH2):
                                nc.tensor.matmul(
                                    ps[:],
                                    dw1[:, m * KH2 * 128 + kk * 128:
                                        m * KH2 * 128 + kk * 128 + 128],
                                    hmT[kk][:, n2 * 512:(n2 + 1) * 512],
                                    start=(kk == 0), stop=(kk == KH2 - 1))
                            nc.scalar.activation(
                                dT[m][:, n2 * 512:(n2 + 1) * 512], ps[:],
                                SIG, bias=db1t[:, m:m + 1])

                    if 'C' in stages:
                        stageC(0)
                    if 'B' in stages:
                        for r in range(5, 8):
                            trans(r, hmasks.pop(r))
                    if 'C' in stages:
                        stageC(1)

                    if 'D' in stages:
                        for m in range(M4):
                            dw0s = dw0p.tile([128, KH1 * 128], bf16,
                                             tag="dw0s")
                            nc.sync.dma_start(dw0s[:], DW0R[m])
                            om = outp.tile([128, BC], f32, tag="om")
                            for n2 in range(2):
                                ps = psp.tile([128, 512], f32)
                                for kk in range(KH1):
                                    nc.tensor.matmul(
                                        ps[:], dw0s[:, kk * 128:(kk + 1) * 128],
                                        dT[kk][:, n2 * 512:(n2 + 1) * 512],
                                        start=(kk == 0), stop=(kk == KH1 - 1))
                                nc.vector.tensor_scalar_add(
                                    om[:, n2 * 512:(n2 + 1) * 512], ps[:],
                                    db0t[:, m:m + 1])
                            nc.scalar.dma_start(OUTT[m], om[:])
    nc.compile()
    return nc


_NC_CACHE = None


def _get_nc():
    global _NC_CACHE
    if _NC_CACHE is None:
        _NC_CACHE = _build()
    return _NC_CACHE


def _build_looped(loop_k: int):
    return _build(loop_k)


def make_in_maps(x, enc_W0, enc_b0, enc_W1, enc_b1, dec_W1, dec_b1, dec_W0,
                 dec_b0):
    def bf(a):
        return np.asarray(a, np.float32).astype(BF)

    w0r = bf(enc_W0).reshape(KD, 128, M1, 128) \
        .transpose(2, 1, 0, 3).reshape(M1, 128, KD * 128)
    w1r = bf(enc_W1).reshape(KH1, 128, 2, 512) \
        .transpose(2, 1, 0, 3).reshape(2, 128, KH1 * 512)
    dw1r = bf(dec_W1).reshape(KH2, 128, M3, 128) \
        .transpose(2, 1, 0, 3).reshape(M3, 128, KH2 * 128)
    dw0r = bf(dec_W0).reshape(KH1, 128, M4, 128) \
        .transpose(2, 1, 0, 3).reshape(M4, 128, KH1 * 128)
    b1r = bf(enc_b1).reshape(1, H2)
    b0r = np.ascontiguousarray(enc_b0.reshape(M1, 128).T, dtype=np.float32)
    db1r = np.ascontiguousarray(dec_b1.reshape(M3, 128).T, dtype=np.float32)
    db0r = np.ascontiguousarray(dec_b0.reshape(M4, 128).T, dtype=np.float32)
    ident = np.eye(128, dtype=np.float32).astype(BF)
    shared = dict(W0R=np.ascontiguousarray(w0r), W1R=np.ascontiguousarray(w1r),
                  DW1R=np.ascontiguousarray(dw1r),
                  DW0R=np.ascontiguousarray(dw0r), B1R=b1r, B0R=b0r,
                  DB1R=db1r, DB0R=db0r, IDENT=ident,
                  ONESR=np.ones((1, 128), dtype=np.float32).astype(BF))
    xr = bf(x)
    in_maps = []
    for c in range(NCORES):
        shard = xr[c * BC:(c + 1) * BC]          # [BC, D] bf16
        xt = np.ascontiguousarray(
            shard.T.reshape(KD, 128, BC).transpose(1, 0, 2)
        ).reshape(128, KD * BC)
        in_maps.append(dict(shared, XTR=xt))
    return in_maps


def kernel(**inputs) -> np.ndarray:
    from concourse import bass_utils
    nc = _get_nc()
    in_maps = make_in_maps(**inputs)
    res = bass_utils.run_bass_kernel_spmd(nc, in_maps,
                                          core_ids=list(range(NCORES)))
    outs = []
    for c in range(NCORES):
        ot = res.results[c]["OUTT"]              # [M4, 128, BC]
        outs.append(ot.reshape(D, BC).T)         # [BC, D]
    return np.ascontiguousarray(np.concatenate(outs, axis=0), dtype=np.float32)


# revision 7
# speedup vs baseline: 1.2091x; 1.0028x over previous
"""Trainium2 Bass kernel for nn_AE_29171417875247 (k-sparse autoencoder with
top-k masking).

  h1 = sigmoid(x @ enc_W0 + enc_b0)        [B, 2048]
  h2 = sigmoid(h1 @ enc_W1 + enc_b1)       [B, 1024]
  h2 = keep top-51 per row, zero rest      (k = 1024 * 0.05)
  d  = sigmoid(h2 @ dec_W1 + dec_b1)       [B, 2048]
  out = d @ dec_W0 + dec_b0                [B, 4096]

Data-parallel across 8 NeuronCores: each core owns 1024 rows of the batch
and the full (replicated) weights. All matmul operands are bf16 (cast on
host, round-to-nearest-even); PSUM accumulation is fp32, and the sigmoid
outputs feeding top-k stay fp32 so the top-51 selection is (near-)exact.
Host-emulated end-to-end bf16 error vs the f32 reference: 4.6e-3.

Per-core pipeline (order chosen so the serial DVE top-k chains hide under
PE-heavy phases; stage A is split 640/384 rows so both top-k batches get a
long PE window):
  A0: h1T rows 0-639   (W0 streamed, xt resident)
  B0-4: MM2+sigmoid rows 0-639 -> 5 DVE top-k chains run during A1
  A1: h1T rows 640-1023 (W0 re-streamed, ~82us of PE to cover the DVE)
  T0-4: PE-transpose hmask tiles 0-4
  B5-7: MM2 rows 640-1023 -> 3 more DVE chains, hidden under C0
  C0: dT[:, 0:512]   = sigmoid(dW1.T @ hmaskT[:, 0:512])
  T5-7, C1: remaining transposes + dT[:, 512:1024]
  D:  outT = dW0.T @ dT + db0 -> DRAM (dw0 streamed once)
"""
import sys
sys.path.insert(0, '/opt/trn_rl_repo')
import numpy as np
import ml_dtypes

BF = ml_dtypes.bfloat16

B, D, H1, H2 = 8192, 4096, 2048, 1024
NCORES = 8
BC = B // NCORES          # rows per core = 1024
K_TOP = 51                # int(H2 * 0.05)
KD = D // 128             # 32 k-chunks for MM1
KH1 = H1 // 128           # 16
KH2 = H2 // 128           # 8
M1 = H1 // 128            # 16 h1 tiles
M3 = H1 // 128            # 16 dT tiles
M4 = D // 128             # 32 out tiles
ROWS_A0 = 512             # stage-A row split: 4 top-k tiles then 4
ROWS_A1 = BC - ROWS_A0    # 512


def _build(loop_k: int = 1, stages: str = 'ABCD'):
    import contextlib
    import concourse.bacc as bacc
    import concourse.mybir as mybir
    import concourse.tile as tile

    f32 = mybir.dt.float32
    bf16 = mybir.dt.bfloat16
    SIG = mybir.ActivationFunctionType.Sigmoid

    nc = bacc.Bacc("TRN2", target_bir_lowering=False, debug=False)
    XTR = nc.dram_tensor("XTR", (128, KD * BC), bf16, kind="ExternalInput").ap()
    W0R = nc.dram_tensor("W0R", (M1, 128, KD * 128), bf16,
                         kind="ExternalInput").ap()
    W1R = nc.dram_tensor("W1R", (2, 128, KH1 * 512), bf16,
                         kind="ExternalInput").ap()
    DW1R = nc.dram_tensor("DW1R", (M3, 128, KH2 * 128), bf16,
                          kind="ExternalInput").ap()
    DW0R = nc.dram_tensor("DW0R", (M4, 128, KH1 * 128), bf16,
                          kind="ExternalInput").ap()
    B1R = nc.dram_tensor("B1R", (1, H2), bf16, kind="ExternalInput").ap()
    B0R = nc.dram_tensor("B0R", (128, M1), f32, kind="ExternalInput").ap()
    DB1R = nc.dram_tensor("DB1R", (128, M3), f32, kind="ExternalInput").ap()
    DB0R = nc.dram_tensor("DB0R", (128, M4), f32, kind="ExternalInput").ap()
    IDENT = nc.dram_tensor("IDENT", (128, 128), bf16, kind="ExternalInput").ap()
    ONESR = nc.dram_tensor("ONESR", (1, 128), bf16, kind="ExternalInput").ap()
    OUTT = nc.dram_tensor("OUTT", (M4, 128, BC), f32, kind="ExternalOutput").ap()

    with tile.TileContext(nc) as tc:
        loop_cm = tc.For_i(0, loop_k, 1) if loop_k > 1 else contextlib.nullcontext()
        with loop_cm, \
             tc.tile_pool(name="biasp", bufs=1) as biasp, \
             tc.tile_pool(name="cstp", bufs=1) as cstp, \
             tc.tile_pool(name="psum", bufs=6, space="PSUM") as psp, \
             tc.tile_pool(name="tps", bufs=2, space="PSUM") as tpsp:
            # merged per-partition biases: [b0 | db1 | db0] (fp32, act bias)
            biases = biasp.tile([128, M1 + M3 + M4], f32)
            nc.sync.dma_start(biases[:, 0:M1], B0R)
            nc.sync.dma_start(biases[:, M1:M1 + M3], DB1R)
            nc.sync.dma_start(biases[:, M1 + M3:], DB0R)
            b0t = biases[:, 0:M1]
            db1t = biases[:, M1:M1 + M3]
            db0t = biases[:, M1 + M3:M1 + M3 + M4]

            with tc.tile_pool(name="h1T", bufs=1) as h1Tp, \
                 tc.tile_pool(name="w1", bufs=1) as w1p, \
                 tc.tile_pool(name="hmT", bufs=1) as hmTp, \
                 tc.tile_pool(name="h2", bufs=3) as h2p, \
                 tc.tile_pool(name="tk", bufs=2) as tkp, \
                 tc.tile_pool(name="hmp", bufs=5) as hmp, \
                 tc.tile_pool(name="mx8", bufs=2) as mxp:
                # h1T: [128 H1-part x (m-tile, 1024 rows)] bf16, SBUF-resident
                h1T = h1Tp.tile([128, M1 * BC], bf16)
                w1h = w1p.tile([128, 2 * KH1 * 512], bf16)
                # hmaskT, kk-major: [128 H2-part x (kk, 1024 rows)]
                hmTt = hmTp.tile([128, KH2 * BC], bf16)
                ident = cstp.tile([128, 128], bf16)
                b1t = cstp.tile([1, H2], bf16)
                ones1 = cstp.tile([1, 128], bf16)

                def mm2_topk(r):
                    """MM2 + sigmoid + top-51 for row tile r (128 rows).
                    PE: 2x(16 mm + rank-1 bias mm); DVE: 7x(max8+match
                    replace); Pool: hmask = h2 - zap (cast to bf16)."""
                    h2r = h2p.tile([128, H2], f32, tag="h2")
                    for n in range(2):
                        ps = psp.tile([128, 512], f32)
                        for kk in range(KH1):
                            nc.tensor.matmul(
                                ps[:],
                                h1T[:, kk * BC + r * 128:
                                    kk * BC + r * 128 + 128],
                                w1h[:, (n * KH1 + kk) * 512:
                                    (n * KH1 + kk) * 512 + 512],
                                start=(kk == 0), stop=False)
                        nc.tensor.matmul(ps[:], ones1[:],
                                         b1t[:, n * 512:(n + 1) * 512],
                                         start=False, stop=True)
                        nc.scalar.activation(h2r[:, n * 512:(n + 1) * 512],
                                             ps[:], SIG)
                    zap = tkp.tile([128, H2], f32, tag="zap")
                    cur = h2r
                    for it in range(7):
                        mx = mxp.tile([128, 8], f32, tag="mx")
                        nc.vector.max(mx[:], cur[:])
                        if it == 6:
                            nc.vector.memset(mx[:, 3:8], 0.0)
                        nc.vector.match_replace(
                            out=zap[:], in_to_replace=mx[:],
                            in_values=cur[:], imm_value=0.0)
                        cur = zap
                    hmask = hmp.tile([128, H2], bf16, tag="hmask")
                    nc.gpsimd.tensor_sub(hmask[:], h2r[:], zap[:])
                    return hmask

                def trans(r, hm):
                    # 8 PE transposes into one psum bank, then a single
                    # strided copy into the kk-major hmTt layout
                    pst = tpsp.tile([128, H2], bf16, name="pst")
                    p3 = pst[:].rearrange("p (kk j) -> p kk j", kk=KH2)
                    for kk in range(KH2):
                        nc.tensor.transpose(
                            p3[:, kk, :], hm[:, kk * 128:(kk + 1) * 128],
                            ident[:])
                    dst = hmTt[:].rearrange(
                        "p (kk bc) -> p kk bc", kk=KH2)[:, :,
                                                        r * 128:(r + 1) * 128]
                    nc.scalar.copy(dst, p3)

                hmasks = {}
                with tc.tile_pool(name="xt", bufs=1) as xtp, \
                     tc.tile_pool(name="w0", bufs=2) as w0p:
                    xt = xtp.tile([128, KD * BC], bf16)
                    # rows 0..ROWS_A0 of every k-chunk first (A0's working set)
                    for k in range(KD):
                        nc.scalar.dma_start(xt[:, k * BC:k * BC + ROWS_A0],
                                            XTR[:, k * BC:k * BC + ROWS_A0])
                    nc.scalar.dma_start(ident[:], IDENT)
                    nc.scalar.dma_start(b1t[:], B1R)
                    nc.scalar.dma_start(ones1[:], ONESR)

                    def stageA(row_off, nrows, half, hook=None):
                        for m in range(M1):
                            if hook is not None:
                                hook(m)
                            w0s = w0p.tile([128, KD * 128], bf16, tag="w0s",
                                           name=f"w0s{half}_{m}")
                            nc.sync.dma_start(w0s[:], W0R[m])
                            o = 0
                            while o < nrows:
                                w = min(512, nrows - o)
                                ps = psp.tile([128, 512], f32)
                                for k in range(KD):
                                    c = k * BC + row_off + o
                                    nc.tensor.matmul(
                                        ps[:, 0:w], w0s[:, k * 128:(k + 1) * 128],
                                        xt[:, c:c + w],
                                        start=(k == 0), stop=(k == KD - 1))
                                nc.scalar.activation(
                                    h1T[:, m * BC + row_off + o:
                                        m * BC + row_off + o + w],
                                    ps[:, 0:w], SIG, bias=b0t[:, m:m + 1])
                                o += w

                    HF = KH1 * 256

                    def prefetch(m):
                        # after A0 m-tile m's act: 3 xt-n1 chunks, then a W1
                        # quarter every 4th m (scalar ring, staggered)
                        for k in range(3 * m, min(3 * m + 3, KD)):
                            nc.scalar.dma_start(
                                xt[:, k * BC + ROWS_A0:k * BC + BC],
                                XTR[:, k * BC + ROWS_A0:k * BC + BC])
                        if m in (4, 7, 10, 13):
                            q = (m - 4) // 3
                            nc.scalar.dma_start(
                                w1h[:, q * HF:(q + 1) * HF],
                                W1R[q // 2, :, (q % 2) * HF:(q % 2 + 1) * HF])

                    if 'A' in stages:
                        stageA(0, ROWS_A0, 0, hook=prefetch)
                    else:
                        for m in range(M1):
                            prefetch(m)
                    if 'B' in stages:
                        for r in range(4):
                            hmasks[r] = mm2_topk(r)
                    if 'A' in stages:
                        stageA(ROWS_A0, ROWS_A1, 1)

                # xt + W0 pools closed: their SBUF is reused below
                with tc.tile_pool(name="dw1", bufs=1) as dw1p, \
                     tc.tile_pool(name="dT", bufs=1) as dTp, \
                     tc.tile_pool(name="dw0", bufs=3) as dw0p, \
                     tc.tile_pool(name="outp", bufs=4) as outp:
                    dw1 = dw1p.tile([128, M3 * KH2 * 128], bf16)
                    for m in range(M3):
                        nc.sync.dma_start(
                            dw1[:, m * KH2 * 128:(m + 1) * KH2 * 128],
                            DW1R[m])
                    dT = [dTp.tile([128, BC], bf16, tag=f"dT{m}",
                                   name=f"dT{m}") for m in range(M3)]

                    if 'B' in stages:
                        for r in range(4, 8):
                            hmasks[r] = mm2_topk(r)
                        for r in range(5):
                            trans(r, hmasks.pop(r))

                    def stageC(n2):
                        for m in range(M3):
                            ps = psp.tile([128, 512], f32)
                            for kk in range(KH2):
                                nc.tensor.matmul(
                                    ps[:],
                                    dw1[:, m * KH2 * 128 + kk * 128:
                                        m * KH2 * 128 + kk * 128 + 128],
                                    hmTt[:, kk * BC + n2 * 512:
                                         kk * BC + n2 * 512 + 512],
                                    start=(kk == 0), stop=(kk == KH2 - 1))
                            nc.scalar.activation(
                                dT[m][:, n2 * 512:(n2 + 1) * 512], ps[:],
                                SIG, bias=db1t[:, m:m + 1])

                    def stageD(n2):
                        # one 512-row sweep over all 32 out tiles; dw0 is
                        # re-streamed per sweep (DMA has headroom, and this
                        # lets sweep 0 start right after C0 so the last
                        # top-k chains hide under it)
                        for m in range(M4):
                            dw0s = dw0p.tile([128, KH1 * 128], bf16,
                                             tag="dw0s")
                            nc.sync.dma_start(dw0s[:], DW0R[m])
                            om = outp.tile([128, 512], f32, tag="om")
                            ps = psp.tile([128, 512], f32)
                            for kk in range(KH1):
                                nc.tensor.matmul(
                                    ps[:], dw0s[:, kk * 128:(kk + 1) * 128],
                                    dT[kk][:, n2 * 512:(n2 + 1) * 512],
                                    start=(kk == 0), stop=(kk == KH1 - 1))
                            nc.vector.tensor_scalar_add(
                                om[:], ps[:], db0t[:, m:m + 1])
                            nc.scalar.dma_start(
                                OUTT[m][:, n2 * 512:(n2 + 1) * 512], om[:])

                    if 'C' in stages:
                        stageC(0)
                    if 'D' in stages:
                        stageD(0)
                    if 'B' in stages:
                        for r in range(5, 8):
                            trans(r, hmasks.pop(r))
                    if 'C' in stages:
                        stageC(1)
                    if 'D' in stages:
                        stageD(1)
    nc.compile()
    return nc


_NC_CACHE = None


def _get_nc():
    global _NC_CACHE
    if _NC_CACHE is None:
        _NC_CACHE = _build()
    return _NC_CACHE


def _build_looped(loop_k: int):
    return _build(loop_k)


def make_in_maps(x, enc_W0, enc_b0, enc_W1, enc_b1, dec_W1, dec_b1, dec_W0,
                 dec_b0):
    def bf(a):
        return np.asarray(a, np.float32).astype(BF)

    w0r = bf(enc_W0).reshape(KD, 128, M1, 128) \
        .transpose(2, 1, 0, 3).reshape(M1, 128, KD * 128)
    w1r = bf(enc_W1).reshape(KH1, 128, 2, 512) \
        .transpose(2, 1, 0, 3).reshape(2, 128, KH1 * 512)
    dw1r = bf(dec_W1).reshape(KH2, 128, M3, 128) \
        .transpose(2, 1, 0, 3).reshape(M3, 128, KH2 * 128)
    dw0r = bf(dec_W0).reshape(KH1, 128, M4, 128) \
        .transpose(2, 1, 0, 3).reshape(M4, 128, KH1 * 128)
    b1r = bf(enc_b1).reshape(1, H2)
    b0r = np.ascontiguousarray(enc_b0.reshape(M1, 128).T, dtype=np.float32)
    db1r = np.ascontiguousarray(dec_b1.reshape(M3, 128).T, dtype=np.float32)
    db0r = np.ascontiguousarray(dec_b0.reshape(M4, 128).T, dtype=np.float32)
    ident = np.eye(128, dtype=np.float32).astype(BF)
    shared = dict(W0R=np.ascontiguousarray(w0r), W1R=np.ascontiguousarray(w1r),
                  DW1R=np.ascontiguousarray(dw1r),
                  DW0R=np.ascontiguousarray(dw0r), B1R=b1r, B0R=b0r,
                  DB1R=db1r, DB0R=db0r, IDENT=ident,
                  ONESR=np.ones((1, 128), dtype=np.float32).astype(BF))
    xr = bf(x)
    in_maps = []
    for c in range(NCORES):
        shard = xr[c * BC:(c + 1) * BC]          # [BC, D] bf16
        xt = np.ascontiguousarray(
            shard.T.reshape(KD, 128, BC).transpose(1, 0, 2)
        ).reshape(128, KD * BC)
        in_maps.append(dict(shared, XTR=xt))
    return in_maps


def kernel(**inputs) -> np.ndarray:
    from concourse import bass_utils
    nc = _get_nc()
    in_maps = make_in_maps(**inputs)
    res = bass_utils.run_bass_kernel_spmd(nc, in_maps,
                                          core_ids=list(range(NCORES)))
    outs = []
    for c in range(NCORES):
        ot = res.results[c]["OUTT"]              # [M4, 128, BC]
        outs.append(ot.reshape(D, BC).T)         # [BC, D]
    return np.ascontiguousarray(np.concatenate(outs, axis=0), dtype=np.float32)


# revision 8
# speedup vs baseline: 1.5366x; 1.2708x over previous
"""Trainium2 Bass kernel for nn_AE_29171417875247 (k-sparse autoencoder with
top-k masking).

  h1 = sigmoid(x @ enc_W0 + enc_b0)        [B, 2048]
  h2 = sigmoid(h1 @ enc_W1 + enc_b1)       [B, 1024]
  h2 = keep top-51 per row, zero rest      (k = 1024 * 0.05)
  d  = sigmoid(h2 @ dec_W1 + dec_b1)       [B, 2048]
  out = d @ dec_W0 + dec_b0                [B, 4096]

Data-parallel across 8 NeuronCores: each core owns 1024 rows of the batch
and the full (replicated) weights. All matmul operands are bf16 (cast on
host, round-to-nearest-even); PSUM accumulation is fp32, and the sigmoid
outputs feeding top-k stay fp32 so the top-51 selection is (near-)exact.
Host-emulated end-to-end bf16 error vs the f32 reference: 4.6e-3.

Per-core pipeline (order chosen so the serial DVE top-k chains hide under
PE-heavy phases; stage A is split 640/384 rows so both top-k batches get a
long PE window):
  A0: h1T rows 0-639   (W0 streamed, xt resident)
  B0-4: MM2+sigmoid rows 0-639 -> 5 DVE top-k chains run during A1
  A1: h1T rows 640-1023 (W0 re-streamed, ~82us of PE to cover the DVE)
  T0-4: PE-transpose hmask tiles 0-4
  B5-7: MM2 rows 640-1023 -> 3 more DVE chains, hidden under C0
  C0: dT[:, 0:512]   = sigmoid(dW1.T @ hmaskT[:, 0:512])
  T5-7, C1: remaining transposes + dT[:, 512:1024]
  D:  outT = dW0.T @ dT + db0 -> DRAM (dw0 streamed once)
"""
import sys
sys.path.insert(0, '/opt/trn_rl_repo')
import numpy as np
import ml_dtypes

BF = ml_dtypes.bfloat16

B, D, H1, H2 = 8192, 4096, 2048, 1024
NCORES = 8
BC = B // NCORES          # rows per core = 1024
K_TOP = 51                # int(H2 * 0.05)
KD = D // 128             # 32 k-chunks for MM1
KH1 = H1 // 128           # 16
KH2 = H2 // 128           # 8
M1 = H1 // 128            # 16 h1 tiles
M3 = H1 // 128            # 16 dT tiles
M4 = D // 128             # 32 out tiles
ROWS_A0 = 512             # stage-A row split: 4 top-k tiles then 4
ROWS_A1 = BC - ROWS_A0    # 512
KDP = KD // 2             # 16 DoubleRow k-pairs for MM1
KH2P = KH2 // 2           # 4 DoubleRow k-pairs for MM3
W0_SCALE = 64.0           # W0 pre-scaled into fp8 range; act un-scales
DW1_SCALE = 256.0


def _build(loop_k: int = 1, stages: str = 'ABCD'):
    import contextlib
    import concourse.bacc as bacc
    import concourse.mybir as mybir
    import concourse.tile as tile

    f32 = mybir.dt.float32
    bf16 = mybir.dt.bfloat16
    f8 = mybir.dt.float8e4
    DR = mybir.MatmulPerfMode.DoubleRow
    SIG = mybir.ActivationFunctionType.Sigmoid

    nc = bacc.Bacc("TRN2", target_bir_lowering=False, debug=False)
    XTR = nc.dram_tensor("XTR", (128, KD * BC), f8, kind="ExternalInput").ap()
    W0R = nc.dram_tensor("W0R", (M1, 128, KD * 128), f8,
                         kind="ExternalInput").ap()
    W1R = nc.dram_tensor("W1R", (2, 128, KH1 * 512), bf16,
                         kind="ExternalInput").ap()
    DW1R = nc.dram_tensor("DW1R", (M3, 128, KH2 * 128), f8,
                          kind="ExternalInput").ap()
    DW0R = nc.dram_tensor("DW0R", (M4, 128, KH1 * 128), bf16,
                          kind="ExternalInput").ap()
    B1R = nc.dram_tensor("B1R", (1, H2), bf16, kind="ExternalInput").ap()
    B0R = nc.dram_tensor("B0R", (128, M1), f32, kind="ExternalInput").ap()
    DB1R = nc.dram_tensor("DB1R", (128, M3), f32, kind="ExternalInput").ap()
    DB0R = nc.dram_tensor("DB0R", (128, M4), f32, kind="ExternalInput").ap()
    IDENT = nc.dram_tensor("IDENT", (128, 128), bf16, kind="ExternalInput").ap()
    ONESR = nc.dram_tensor("ONESR", (1, 128), bf16, kind="ExternalInput").ap()
    OUTT = nc.dram_tensor("OUTT", (M4, 128, BC), f32, kind="ExternalOutput").ap()

    with tile.TileContext(nc) as tc:
        loop_cm = tc.For_i(0, loop_k, 1) if loop_k > 1 else contextlib.nullcontext()
        with loop_cm, \
             tc.tile_pool(name="biasp", bufs=1) as biasp, \
             tc.tile_pool(name="cstp", bufs=1) as cstp, \
             tc.tile_pool(name="psum", bufs=6, space="PSUM") as psp, \
             tc.tile_pool(name="tps", bufs=2, space="PSUM") as tpsp:
            # merged per-partition biases: [b0 | db1 | db0] (fp32, act bias)
            biases = biasp.tile([128, M1 + M3 + M4], f32)
            nc.sync.dma_start(biases[:, 0:M1], B0R)
            nc.sync.dma_start(biases[:, M1:M1 + M3], DB1R)
            nc.sync.dma_start(biases[:, M1 + M3:], DB0R)
            b0t = biases[:, 0:M1]
            db1t = biases[:, M1:M1 + M3]
            db0t = biases[:, M1 + M3:M1 + M3 + M4]

            with tc.tile_pool(name="h1T", bufs=1) as h1Tp, \
                 tc.tile_pool(name="w1", bufs=1) as w1p, \
                 tc.tile_pool(name="hmT", bufs=1) as hmTp, \
                 tc.tile_pool(name="h2", bufs=3) as h2p, \
                 tc.tile_pool(name="tk", bufs=2) as tkp, \
                 tc.tile_pool(name="hmp", bufs=5) as hmp, \
                 tc.tile_pool(name="mx8", bufs=2) as mxp:
                # h1T: [128 H1-part x (m-tile, 1024 rows)] bf16, SBUF-resident
                h1T = h1Tp.tile([128, M1 * BC], bf16)
                w1h = w1p.tile([128, 2 * KH1 * 512], bf16)
                # hmaskT, kk-major: [128 H2-part x (kk, 1024 rows)] fp8
                hmTt = hmTp.tile([128, KH2 * BC], f8)
                ident = cstp.tile([128, 128], bf16)
                b1t = cstp.tile([1, H2], bf16)
                ones1 = cstp.tile([1, 128], bf16)

                def mm2_topk(r):
                    """MM2 + sigmoid + top-51 for row tile r (128 rows).
                    PE: 2x(16 mm + rank-1 bias mm); DVE: 7x(max8+match
                    replace); Pool: hmask = h2 - zap (cast to bf16)."""
                    h2r = h2p.tile([128, H2], f32, tag="h2")
                    for n in range(2):
                        ps = psp.tile([128, 512], f32)
                        for kk in range(KH1):
                            nc.tensor.matmul(
                                ps[:],
                                h1T[:, kk * BC + r * 128:
                                    kk * BC + r * 128 + 128],
                                w1h[:, (n * KH1 + kk) * 512:
                                    (n * KH1 + kk) * 512 + 512],
                                start=(kk == 0), stop=False)
                        nc.tensor.matmul(ps[:], ones1[:],
                                         b1t[:, n * 512:(n + 1) * 512],
                                         start=False, stop=True)
                        nc.scalar.activation(h2r[:, n * 512:(n + 1) * 512],
                                             ps[:], SIG)
                    zap = tkp.tile([128, H2], f32, tag="zap")
                    cur = h2r
                    for it in range(7):
                        mx = mxp.tile([128, 8], f32, tag="mx")
                        nc.vector.max(mx[:], cur[:])
                        if it == 6:
                            nc.vector.memset(mx[:, 3:8], 0.0)
                        nc.vector.match_replace(
                            out=zap[:], in_to_replace=mx[:],
                            in_values=cur[:], imm_value=0.0)
                        cur = zap
                    hmask = hmp.tile([128, H2], bf16, tag="hmask")
                    nc.gpsimd.tensor_sub(hmask[:], h2r[:], zap[:])
                    return hmask

                def trans(r, hm):
                    # 8 PE transposes into one psum bank, then a single
                    # strided copy into the kk-major hmTt layout
                    pst = tpsp.tile([128, H2], bf16, name="pst")
                    p3 = pst[:].rearrange("p (kk j) -> p kk j", kk=KH2)
                    for kk in range(KH2):
                        nc.tensor.transpose(
                            p3[:, kk, :], hm[:, kk * 128:(kk + 1) * 128],
                            ident[:])
                    dst = hmTt[:].rearrange(
                        "p (kk bc) -> p kk bc", kk=KH2)[:, :,
                                                        r * 128:(r + 1) * 128]
                    nc.scalar.copy(dst, p3)

                hmasks = {}
                with tc.tile_pool(name="xt", bufs=1) as xtp, \
                     tc.tile_pool(name="w0", bufs=2) as w0p:
                    xt = xtp.tile([128, KD * BC], f8)
                    # rows 0..ROWS_A0 of every k-chunk first (A0's working set)
                    for k in range(KD):
                        nc.scalar.dma_start(xt[:, k * BC:k * BC + ROWS_A0],
                                            XTR[:, k * BC:k * BC + ROWS_A0])
                    nc.scalar.dma_start(ident[:], IDENT)
                    nc.scalar.dma_start(b1t[:], B1R)
                    nc.scalar.dma_start(ones1[:], ONESR)

                    xt4 = xt[:].rearrange("p (j i bc) -> p j i bc",
                                          j=KDP, i=2)

                    def stageA(row_off, nrows, half, hook=None):
                        for m in range(M1):
                            if hook is not None:
                                hook(m)
                            w0s = w0p.tile([128, KD * 128], f8, tag="w0s",
                                           name=f"w0s{half}_{m}")
                            nc.sync.dma_start(w0s[:], W0R[m])
                            w0s4 = w0s[:].rearrange(
                                "p (j i c) -> p j i c", j=KDP, i=2)
                            o = 0
                            while o < nrows:
                                w = min(512, nrows - o)
                                ps = psp.tile([128, 512], f32)
                                c = row_off + o
                                for j in range(KDP):
                                    nc.tensor.matmul(
                                        ps[:, 0:w], w0s4[:, j],
                                        xt4[:, j, :, c:c + w],
                                        start=(j == 0), stop=(j == KDP - 1),
                                        perf_mode=DR)
                                nc.scalar.activation(
                                    h1T[:, m * BC + row_off + o:
                                        m * BC + row_off + o + w],
                                    ps[:, 0:w], SIG, bias=b0t[:, m:m + 1],
                                    scale=1.0 / W0_SCALE)
                                o += w

                    HF = KH1 * 256

                    def prefetch(m):
                        # after A0 m-tile m's act: 3 xt-n1 chunks, then a W1
                        # quarter every 4th m (scalar ring, staggered)
                        for k in range(3 * m, min(3 * m + 3, KD)):
                            nc.scalar.dma_start(
                                xt[:, k * BC + ROWS_A0:k * BC + BC],
                                XTR[:, k * BC + ROWS_A0:k * BC + BC])
                        if m in (4, 7, 10, 13):
                            q = (m - 4) // 3
                            nc.scalar.dma_start(
                                w1h[:, q * HF:(q + 1) * HF],
                                W1R[q // 2, :, (q % 2) * HF:(q % 2 + 1) * HF])

                    if 'A' in stages:
                        stageA(0, ROWS_A0, 0, hook=prefetch)
                    else:
                        for m in range(M1):
                            prefetch(m)
                    if 'B' in stages:
                        for r in range(4):
                            hmasks[r] = mm2_topk(r)
                    if 'A' in stages:
                        stageA(ROWS_A0, ROWS_A1, 1)

                # xt + W0 pools closed: their SBUF is reused below
                with tc.tile_pool(name="dw1", bufs=1) as dw1p, \
                     tc.tile_pool(name="dT", bufs=1) as dTp, \
                     tc.tile_pool(name="dw0", bufs=3) as dw0p, \
                     tc.tile_pool(name="outp", bufs=4) as outp:
                    dw1 = dw1p.tile([128, M3 * KH2 * 128], f8)
                    for m in range(M3):
                        nc.sync.dma_start(
                            dw1[:, m * KH2 * 128:(m + 1) * KH2 * 128],
                            DW1R[m])
                    dT = [dTp.tile([128, BC], bf16, tag=f"dT{m}",
                                   name=f"dT{m}") for m in range(M3)]

                    if 'B' in stages:
                        for r in range(4, 8):
                            hmasks[r] = mm2_topk(r)
                        for r in range(5):
                            trans(r, hmasks.pop(r))

                    hmT4 = hmTt[:].rearrange("p (j i bc) -> p j i bc",
                                             j=KH2P, i=2)
                    dw14 = dw1[:].rearrange("p (m j i c) -> p m j i c",
                                            m=M3, j=KH2P, i=2)

                    def stageC(n2):
                        for m in range(M3):
                            ps = psp.tile([128, 512], f32)
                            for j in range(KH2P):
                                nc.tensor.matmul(
                                    ps[:], dw14[:, m, j],
                                    hmT4[:, j, :, n2 * 512:(n2 + 1) * 512],
                                    start=(j == 0), stop=(j == KH2P - 1),
                                    perf_mode=DR)
                            nc.scalar.activation(
                                dT[m][:, n2 * 512:(n2 + 1) * 512], ps[:],
                                SIG, bias=db1t[:, m:m + 1],
                                scale=1.0 / DW1_SCALE)

                    def stageD(n2):
                        # one 512-row sweep over all 32 out tiles; dw0 is
                        # re-streamed per sweep (DMA has headroom, and this
                        # lets sweep 0 start right after C0 so the last
                        # top-k chains hide under it)
                        for m in range(M4):
                            dw0s = dw0p.tile([128, KH1 * 128], bf16,
                                             tag="dw0s")
                            nc.sync.dma_start(dw0s[:], DW0R[m])
                            om = outp.tile([128, 512], f32, tag="om")
                            ps = psp.tile([128, 512], f32)
                            for kk in range(KH1):
                                nc.tensor.matmul(
                                    ps[:], dw0s[:, kk * 128:(kk + 1) * 128],
                                    dT[kk][:, n2 * 512:(n2 + 1) * 512],
                                    start=(kk == 0), stop=(kk == KH1 - 1))
                            nc.vector.tensor_scalar_add(
                                om[:], ps[:], db0t[:, m:m + 1])
                            nc.scalar.dma_start(
                                OUTT[m][:, n2 * 512:(n2 + 1) * 512], om[:])

                    if 'C' in stages:
                        stageC(0)
                    if 'D' in stages:
                        stageD(0)
                    if 'B' in stages:
                        for r in range(5, 8):
                            trans(r, hmasks.pop(r))
                    if 'C' in stages:
                        stageC(1)
                    if 'D' in stages:
                        stageD(1)
    nc.compile()
    return nc


_NC_CACHE = None


def _get_nc():
    global _NC_CACHE
    if _NC_CACHE is None:
        _NC_CACHE = _build()
    return _NC_CACHE


def _build_looped(loop_k: int):
    return _build(loop_k)


def make_in_maps(x, enc_W0, enc_b0, enc_W1, enc_b1, dec_W1, dec_b1, dec_W0,
                 dec_b0):
    F8 = ml_dtypes.float8_e4m3

    def bf(a):
        return np.asarray(a, np.float32).astype(BF)

    # fp8 DoubleRow pairing: k-chunks (2j, 2j+1) interleave along the free
    # axis as [j, i, .] with i the sub-chunk
    w0r = (np.asarray(enc_W0, np.float32) * W0_SCALE).astype(F8) \
        .reshape(KDP, 2, 128, M1, 128) \
        .transpose(3, 2, 0, 1, 4).reshape(M1, 128, KD * 128)
    w1r = bf(enc_W1).reshape(KH1, 128, 2, 512) \
        .transpose(2, 1, 0, 3).reshape(2, 128, KH1 * 512)
    dw1r = (np.asarray(dec_W1, np.float32) * DW1_SCALE).astype(F8) \
        .reshape(KH2P, 2, 128, M3, 128) \
        .transpose(3, 2, 0, 1, 4).reshape(M3, 128, KH2 * 128)
    dw0r = bf(dec_W0).reshape(KH1, 128, M4, 128) \
        .transpose(2, 1, 0, 3).reshape(M4, 128, KH1 * 128)
    b1r = bf(enc_b1).reshape(1, H2)
    b0r = np.ascontiguousarray(enc_b0.reshape(M1, 128).T, dtype=np.float32)
    db1r = np.ascontiguousarray(dec_b1.reshape(M3, 128).T, dtype=np.float32)
    db0r = np.ascontiguousarray(dec_b0.reshape(M4, 128).T, dtype=np.float32)
    ident = np.eye(128, dtype=np.float32).astype(BF)
    shared = dict(W0R=np.ascontiguousarray(w0r), W1R=np.ascontiguousarray(w1r),
                  DW1R=np.ascontiguousarray(dw1r),
                  DW0R=np.ascontiguousarray(dw0r), B1R=b1r, B0R=b0r,
                  DB1R=db1r, DB0R=db0r, IDENT=ident,
                  ONESR=np.ones((1, 128), dtype=np.float32).astype(BF))
    xr = np.asarray(x, np.float32).astype(F8)
    in_maps = []
    for c in range(NCORES):
        shard = xr[c * BC:(c + 1) * BC]          # [BC, D] fp8
        xt = np.ascontiguousarray(
            shard.T.reshape(KDP, 2, 128, BC).transpose(2, 0, 1, 3)
        ).reshape(128, KD * BC)
        in_maps.append(dict(shared, XTR=xt))
    return in_maps


def kernel(**inputs) -> np.ndarray:
    from concourse import bass_utils
    nc = _get_nc()
    in_maps = make_in_maps(**inputs)
    res = bass_utils.run_bass_kernel_spmd(nc, in_maps,
                                          core_ids=list(range(NCORES)))
    outs = []
    for c in range(NCORES):
        ot = res.results[c]["OUTT"]              # [M4, 128, BC]
        outs.append(ot.reshape(D, BC).T)         # [BC, D]
    return np.ascontiguousarray(np.concatenate(outs, axis=0), dtype=np.float32)


# revision 9
# speedup vs baseline: 1.5388x; 1.0014x over previous
"""Trainium2 Bass kernel for nn_AE_29171417875247 (k-sparse autoencoder with
top-k masking).

  h1 = sigmoid(x @ enc_W0 + enc_b0)        [B, 2048]
  h2 = sigmoid(h1 @ enc_W1 + enc_b1)       [B, 1024]
  h2 = keep top-51 per row, zero rest      (k = 1024 * 0.05)
  d  = sigmoid(h2 @ dec_W1 + dec_b1)       [B, 2048]
  out = d @ dec_W0 + dec_b0                [B, 4096]

Data-parallel across 8 NeuronCores: each core owns 1024 rows of the batch
and the full (replicated) weights. All matmul operands are bf16 (cast on
host, round-to-nearest-even); PSUM accumulation is fp32, and the sigmoid
outputs feeding top-k stay fp32 so the top-51 selection is (near-)exact.
Host-emulated end-to-end bf16 error vs the f32 reference: 4.6e-3.

Per-core pipeline (order chosen so the serial DVE top-k chains hide under
PE-heavy phases; stage A is split 640/384 rows so both top-k batches get a
long PE window):
  A0: h1T rows 0-639   (W0 streamed, xt resident)
  B0-4: MM2+sigmoid rows 0-639 -> 5 DVE top-k chains run during A1
  A1: h1T rows 640-1023 (W0 re-streamed, ~82us of PE to cover the DVE)
  T0-4: PE-transpose hmask tiles 0-4
  B5-7: MM2 rows 640-1023 -> 3 more DVE chains, hidden under C0
  C0: dT[:, 0:512]   = sigmoid(dW1.T @ hmaskT[:, 0:512])
  T5-7, C1: remaining transposes + dT[:, 512:1024]
  D:  outT = dW0.T @ dT + db0 -> DRAM (dw0 streamed once)
"""
import sys
sys.path.insert(0, '/opt/trn_rl_repo')
import numpy as np
import ml_dtypes

BF = ml_dtypes.bfloat16

B, D, H1, H2 = 8192, 4096, 2048, 1024
NCORES = 8
BC = B // NCORES          # rows per core = 1024
K_TOP = 51                # int(H2 * 0.05)
KD = D // 128             # 32 k-chunks for MM1
KH1 = H1 // 128           # 16
KH2 = H2 // 128           # 8
M1 = H1 // 128            # 16 h1 tiles
M3 = H1 // 128            # 16 dT tiles
M4 = D // 128             # 32 out tiles
ROWS_A0 = 512             # stage-A row split: 4 top-k tiles then 4
ROWS_A1 = BC - ROWS_A0    # 512
KDP = KD // 2             # 16 DoubleRow k-pairs for MM1
KH2P = KH2 // 2           # 4 DoubleRow k-pairs for MM3
W0_SCALE = 64.0           # W0 pre-scaled into fp8 range; act un-scales
DW1_SCALE = 256.0


def _build(loop_k: int = 1, stages: str = 'ABCD'):
    import contextlib
    import concourse.bacc as bacc
    import concourse.mybir as mybir
    import concourse.tile as tile

    f32 = mybir.dt.float32
    bf16 = mybir.dt.bfloat16
    f8 = mybir.dt.float8e4
    DR = mybir.MatmulPerfMode.DoubleRow
    SIG = mybir.ActivationFunctionType.Sigmoid

    nc = bacc.Bacc("TRN2", target_bir_lowering=False, debug=False)
    XTR = nc.dram_tensor("XTR", (128, KD * BC), f8, kind="ExternalInput").ap()
    W0R = nc.dram_tensor("W0R", (M1, 128, KD * 128), f8,
                         kind="ExternalInput").ap()
    W1R = nc.dram_tensor("W1R", (2, 128, KH1 * 512), bf16,
                         kind="ExternalInput").ap()
    DW1R = nc.dram_tensor("DW1R", (M3, 128, KH2 * 128), f8,
                          kind="ExternalInput").ap()
    DW0R = nc.dram_tensor("DW0R", (M4, 128, KH1 * 128), bf16,
                          kind="ExternalInput").ap()
    B1R = nc.dram_tensor("B1R", (1, H2), bf16, kind="ExternalInput").ap()
    B0R = nc.dram_tensor("B0R", (128, M1), f32, kind="ExternalInput").ap()
    DB1R = nc.dram_tensor("DB1R", (128, M3), f32, kind="ExternalInput").ap()
    DB0R = nc.dram_tensor("DB0R", (128, M4), f32, kind="ExternalInput").ap()
    IDENT = nc.dram_tensor("IDENT", (128, 128), bf16, kind="ExternalInput").ap()
    ONESR = nc.dram_tensor("ONESR", (1, 128), bf16, kind="ExternalInput").ap()
    OUTT = nc.dram_tensor("OUTT", (M4, 128, BC), bf16,
                          kind="ExternalOutput").ap()

    with tile.TileContext(nc) as tc:
        loop_cm = tc.For_i(0, loop_k, 1) if loop_k > 1 else contextlib.nullcontext()
        with loop_cm, \
             tc.tile_pool(name="biasp", bufs=1) as biasp, \
             tc.tile_pool(name="cstp", bufs=1) as cstp, \
             tc.tile_pool(name="psum", bufs=6, space="PSUM") as psp, \
             tc.tile_pool(name="tps", bufs=2, space="PSUM") as tpsp:
            # merged per-partition biases: [b0 | db1 | db0] (fp32, act bias)
            biases = biasp.tile([128, M1 + M3 + M4], f32)
            nc.sync.dma_start(biases[:, 0:M1], B0R)
            nc.sync.dma_start(biases[:, M1:M1 + M3], DB1R)
            nc.sync.dma_start(biases[:, M1 + M3:], DB0R)
            b0t = biases[:, 0:M1]
            db1t = biases[:, M1:M1 + M3]
            db0t = biases[:, M1 + M3:M1 + M3 + M4]

            with tc.tile_pool(name="h1T", bufs=1) as h1Tp, \
                 tc.tile_pool(name="w1", bufs=1) as w1p, \
                 tc.tile_pool(name="hmT", bufs=1) as hmTp, \
                 tc.tile_pool(name="h2", bufs=3) as h2p, \
                 tc.tile_pool(name="tk", bufs=2) as tkp, \
                 tc.tile_pool(name="hmp", bufs=5) as hmp, \
                 tc.tile_pool(name="mx8", bufs=2) as mxp:
                # h1T: [128 H1-part x (m-tile, 1024 rows)] bf16, SBUF-resident
                h1T = h1Tp.tile([128, M1 * BC], bf16)
                w1h = w1p.tile([128, 2 * KH1 * 512], bf16)
                # hmaskT, kk-major: [128 H2-part x (kk, 1024 rows)] fp8
                hmTt = hmTp.tile([128, KH2 * BC], f8)
                ident = cstp.tile([128, 128], bf16)
                b1t = cstp.tile([1, H2], bf16)
                ones1 = cstp.tile([1, 128], bf16)

                def mm2_topk(r):
                    """MM2 + sigmoid + top-51 for row tile r (128 rows).
                    PE: 2x(16 mm + rank-1 bias mm); DVE: 7x(max8+match
                    replace); Pool: hmask = h2 - zap (cast to bf16)."""
                    h2r = h2p.tile([128, H2], f32, tag="h2")
                    for n in range(2):
                        ps = psp.tile([128, 512], f32)
                        for kk in range(KH1):
                            nc.tensor.matmul(
                                ps[:],
                                h1T[:, kk * BC + r * 128:
                                    kk * BC + r * 128 + 128],
                                w1h[:, (n * KH1 + kk) * 512:
                                    (n * KH1 + kk) * 512 + 512],
                                start=(kk == 0), stop=False)
                        nc.tensor.matmul(ps[:], ones1[:],
                                         b1t[:, n * 512:(n + 1) * 512],
                                         start=False, stop=True)
                        nc.scalar.activation(h2r[:, n * 512:(n + 1) * 512],
                                             ps[:], SIG)
                    zap = tkp.tile([128, H2], f32, tag="zap")
                    cur = h2r
                    for it in range(7):
                        mx = mxp.tile([128, 8], f32, tag="mx")
                        nc.vector.max(mx[:], cur[:])
                        if it == 6:
                            nc.vector.memset(mx[:, 3:8], 0.0)
                        nc.vector.match_replace(
                            out=zap[:], in_to_replace=mx[:],
                            in_values=cur[:], imm_value=0.0)
                        cur = zap
                    hmask = hmp.tile([128, H2], bf16, tag="hmask")
                    nc.gpsimd.tensor_sub(hmask[:], h2r[:], zap[:])
                    return hmask

                def trans(r, hm):
                    # 8 PE transposes into one psum bank, then a single
                    # strided copy into the kk-major hmTt layout
                    pst = tpsp.tile([128, H2], bf16, name="pst")
                    p3 = pst[:].rearrange("p (kk j) -> p kk j", kk=KH2)
                    for kk in range(KH2):
                        nc.tensor.transpose(
                            p3[:, kk, :], hm[:, kk * 128:(kk + 1) * 128],
                            ident[:])
                    dst = hmTt[:].rearrange(
                        "p (kk bc) -> p kk bc", kk=KH2)[:, :,
                                                        r * 128:(r + 1) * 128]
                    nc.scalar.copy(dst, p3)

                hmasks = {}
                with tc.tile_pool(name="xt", bufs=1) as xtp, \
                     tc.tile_pool(name="w0", bufs=2) as w0p:
                    xt = xtp.tile([128, KD * BC], f8)
                    # rows 0..ROWS_A0 of every k-chunk first (A0's working set)
                    for k in range(KD):
                        nc.scalar.dma_start(xt[:, k * BC:k * BC + ROWS_A0],
                                            XTR[:, k * BC:k * BC + ROWS_A0])
                    nc.scalar.dma_start(ident[:], IDENT)
                    nc.scalar.dma_start(b1t[:], B1R)
                    nc.scalar.dma_start(ones1[:], ONESR)

                    xt4 = xt[:].rearrange("p (j i bc) -> p j i bc",
                                          j=KDP, i=2)

                    def stageA(row_off, nrows, half, hook=None):
                        for m in range(M1):
                            if hook is not None:
                                hook(m)
                            w0s = w0p.tile([128, KD * 128], f8, tag="w0s",
                                           name=f"w0s{half}_{m}")
                            nc.sync.dma_start(w0s[:], W0R[m])
                            w0s4 = w0s[:].rearrange(
                                "p (j i c) -> p j i c", j=KDP, i=2)
                            o = 0
                            while o < nrows:
                                w = min(512, nrows - o)
                                ps = psp.tile([128, 512], f32)
                                c = row_off + o
                                for j in range(KDP):
                                    nc.tensor.matmul(
                                        ps[:, 0:w], w0s4[:, j],
                                        xt4[:, j, :, c:c + w],
                                        start=(j == 0), stop=(j == KDP - 1),
                                        perf_mode=DR)
                                nc.scalar.activation(
                                    h1T[:, m * BC + row_off + o:
                                        m * BC + row_off + o + w],
                                    ps[:, 0:w], SIG, bias=b0t[:, m:m + 1],
                                    scale=1.0 / W0_SCALE)
                                o += w

                    HF = KH1 * 256

                    def prefetch(m):
                        # after A0 m-tile m's act: 3 xt-n1 chunks, then a W1
                        # quarter every 4th m (scalar ring, staggered)
                        for k in range(3 * m, min(3 * m + 3, KD)):
                            nc.scalar.dma_start(
                                xt[:, k * BC + ROWS_A0:k * BC + BC],
                                XTR[:, k * BC + ROWS_A0:k * BC + BC])
                        if m in (4, 7, 10, 13):
                            q = (m - 4) // 3
                            nc.scalar.dma_start(
                                w1h[:, q * HF:(q + 1) * HF],
                                W1R[q // 2, :, (q % 2) * HF:(q % 2 + 1) * HF])

                    if 'A' in stages:
                        stageA(0, ROWS_A0, 0, hook=prefetch)
                    else:
                        for m in range(M1):
                            prefetch(m)
                    if 'B' in stages:
                        for r in range(4):
                            hmasks[r] = mm2_topk(r)
                    if 'A' in stages:
                        stageA(ROWS_A0, ROWS_A1, 1)

                # xt + W0 pools closed: their SBUF is reused below
                with tc.tile_pool(name="dw1", bufs=1) as dw1p, \
                     tc.tile_pool(name="dT", bufs=1) as dTp, \
                     tc.tile_pool(name="dw0", bufs=3) as dw0p, \
                     tc.tile_pool(name="outp", bufs=4) as outp:
                    dw1 = dw1p.tile([128, M3 * KH2 * 128], f8)
                    for m in range(M3):
                        nc.sync.dma_start(
                            dw1[:, m * KH2 * 128:(m + 1) * KH2 * 128],
                            DW1R[m])
                    dT = [dTp.tile([128, BC], bf16, tag=f"dT{m}",
                                   name=f"dT{m}") for m in range(M3)]

                    if 'B' in stages:
                        for r in range(4, 8):
                            hmasks[r] = mm2_topk(r)
                        for r in range(5):
                            trans(r, hmasks.pop(r))

                    hmT4 = hmTt[:].rearrange("p (j i bc) -> p j i bc",
                                             j=KH2P, i=2)
                    dw14 = dw1[:].rearrange("p (m j i c) -> p m j i c",
                                            m=M3, j=KH2P, i=2)

                    def stageC(n2):
                        for m in range(M3):
                            ps = psp.tile([128, 512], f32)
                            for j in range(KH2P):
                                nc.tensor.matmul(
                                    ps[:], dw14[:, m, j],
                                    hmT4[:, j, :, n2 * 512:(n2 + 1) * 512],
                                    start=(j == 0), stop=(j == KH2P - 1),
                                    perf_mode=DR)
                            nc.scalar.activation(
                                dT[m][:, n2 * 512:(n2 + 1) * 512], ps[:],
                                SIG, bias=db1t[:, m:m + 1],
                                scale=1.0 / DW1_SCALE)

                    def stageD(n2):
                        # one 512-row sweep over all 32 out tiles; dw0 is
                        # re-streamed per sweep (DMA has headroom, and this
                        # lets sweep 0 start right after C0 so the last
                        # top-k chains hide under it)
                        for m in range(M4):
                            dw0s = dw0p.tile([128, KH1 * 128], bf16,
                                             tag="dw0s")
                            nc.sync.dma_start(dw0s[:], DW0R[m])
                            om = outp.tile([128, 512], bf16, tag="om")
                            ps = psp.tile([128, 512], f32)
                            for kk in range(KH1):
                                nc.tensor.matmul(
                                    ps[:], dw0s[:, kk * 128:(kk + 1) * 128],
                                    dT[kk][:, n2 * 512:(n2 + 1) * 512],
                                    start=(kk == 0), stop=(kk == KH1 - 1))
                            nc.vector.tensor_scalar_add(
                                om[:], ps[:], db0t[:, m:m + 1])
                            nc.scalar.dma_start(
                                OUTT[m][:, n2 * 512:(n2 + 1) * 512], om[:])

                    if 'C' in stages:
                        stageC(0)
                    if 'D' in stages:
                        stageD(0)
                    if 'B' in stages:
                        for r in range(5, 8):
                            trans(r, hmasks.pop(r))
                    if 'C' in stages:
                        stageC(1)
                    if 'D' in stages:
                        stageD(1)
    nc.compile()
    return nc


_NC_CACHE = None


def _get_nc():
    global _NC_CACHE
    if _NC_CACHE is None:
        _NC_CACHE = _build()
    return _NC_CACHE


def _build_looped(loop_k: int):
    return _build(loop_k)


def make_in_maps(x, enc_W0, enc_b0, enc_W1, enc_b1, dec_W1, dec_b1, dec_W0,
                 dec_b0):
    F8 = ml_dtypes.float8_e4m3

    def bf(a):
        return np.asarray(a, np.float32).astype(BF)

    # fp8 DoubleRow pairing: k-chunks (2j, 2j+1) interleave along the free
    # axis as [j, i, .] with i the sub-chunk
    w0r = (np.asarray(enc_W0, np.float32) * W0_SCALE).astype(F8) \
        .reshape(KDP, 2, 128, M1, 128) \
        .transpose(3, 2, 0, 1, 4).reshape(M1, 128, KD * 128)
    w1r = bf(enc_W1).reshape(KH1, 128, 2, 512) \
        .transpose(2, 1, 0, 3).reshape(2, 128, KH1 * 512)
    dw1r = (np.asarray(dec_W1, np.float32) * DW1_SCALE).astype(F8) \
        .reshape(KH2P, 2, 128, M3, 128) \
        .transpose(3, 2, 0, 1, 4).reshape(M3, 128, KH2 * 128)
    dw0r = bf(dec_W0).reshape(KH1, 128, M4, 128) \
        .transpose(2, 1, 0, 3).reshape(M4, 128, KH1 * 128)
    b1r = bf(enc_b1).reshape(1, H2)
    b0r = np.ascontiguousarray(enc_b0.reshape(M1, 128).T, dtype=np.float32)
    db1r = np.ascontiguousarray(dec_b1.reshape(M3, 128).T, dtype=np.float32)
    db0r = np.ascontiguousarray(dec_b0.reshape(M4, 128).T, dtype=np.float32)
    ident = np.eye(128, dtype=np.float32).astype(BF)
    shared = dict(W0R=np.ascontiguousarray(w0r), W1R=np.ascontiguousarray(w1r),
                  DW1R=np.ascontiguousarray(dw1r),
                  DW0R=np.ascontiguousarray(dw0r), B1R=b1r, B0R=b0r,
                  DB1R=db1r, DB0R=db0r, IDENT=ident,
                  ONESR=np.ones((1, 128), dtype=np.float32).astype(BF))
    xr = np.asarray(x, np.float32).astype(F8)
    in_maps = []
    for c in range(NCORES):
        shard = xr[c * BC:(c + 1) * BC]          # [BC, D] fp8
        xt = np.ascontiguousarray(
            shard.T.reshape(KDP, 2, 128, BC).transpose(2, 0, 1, 3)
        ).reshape(128, KD * BC)
        in_maps.append(dict(shared, XTR=xt))
    return in_maps


def kernel(**inputs) -> np.ndarray:
    from concourse import bass_utils
    nc = _get_nc()
    in_maps = make_in_maps(**inputs)
    res = bass_utils.run_bass_kernel_spmd(nc, in_maps,
                                          core_ids=list(range(NCORES)))
    outs = []
    for c in range(NCORES):
        ot = res.results[c]["OUTT"].astype(np.float32)   # [M4, 128, BC] bf16
        outs.append(ot.reshape(D, BC).T)                 # [BC, D]
    return np.ascontiguousarray(np.concatenate(outs, axis=0), dtype=np.float32)


# revision 10
# speedup vs baseline: 1.5839x; 1.0293x over previous
"""Trainium2 Bass kernel for nn_AE_29171417875247 (k-sparse autoencoder with
top-k masking).

  h1 = sigmoid(x @ enc_W0 + enc_b0)        [B, 2048]
  h2 = sigmoid(h1 @ enc_W1 + enc_b1)       [B, 1024]
  h2 = keep top-51 per row, zero rest      (k = 1024 * 0.05)
  d  = sigmoid(h2 @ dec_W1 + dec_b1)       [B, 2048]
  out = d @ dec_W0 + dec_b0                [B, 4096]

Data-parallel across 8 NeuronCores: each core owns 1024 rows of the batch
and the full (replicated) weights. All matmul operands are bf16 (cast on
host, round-to-nearest-even); PSUM accumulation is fp32, and the sigmoid
outputs feeding top-k stay fp32 so the top-51 selection is (near-)exact.
Host-emulated end-to-end bf16 error vs the f32 reference: 4.6e-3.

Per-core pipeline (order chosen so the serial DVE top-k chains hide under
PE-heavy phases; stage A is split 640/384 rows so both top-k batches get a
long PE window):
  A0: h1T rows 0-639   (W0 streamed, xt resident)
  B0-4: MM2+sigmoid rows 0-639 -> 5 DVE top-k chains run during A1
  A1: h1T rows 640-1023 (W0 re-streamed, ~82us of PE to cover the DVE)
  T0-4: PE-transpose hmask tiles 0-4
  B5-7: MM2 rows 640-1023 -> 3 more DVE chains, hidden under C0
  C0: dT[:, 0:512]   = sigmoid(dW1.T @ hmaskT[:, 0:512])
  T5-7, C1: remaining transposes + dT[:, 512:1024]
  D:  outT = dW0.T @ dT + db0 -> DRAM (dw0 streamed once)
"""
import sys
sys.path.insert(0, '/opt/trn_rl_repo')
import numpy as np
import ml_dtypes

BF = ml_dtypes.bfloat16

B, D, H1, H2 = 8192, 4096, 2048, 1024
NCORES = 8
BC = B // NCORES          # rows per core = 1024
K_TOP = 51                # int(H2 * 0.05)
KD = D // 128             # 32 k-chunks for MM1
KH1 = H1 // 128           # 16
KH2 = H2 // 128           # 8
M1 = H1 // 128            # 16 h1 tiles
M3 = H1 // 128            # 16 dT tiles
M4 = D // 128             # 32 out tiles
ROWS_A0 = 512             # stage-A row split: 4 top-k tiles then 4
ROWS_A1 = BC - ROWS_A0    # 512
KDP = KD // 2             # 16 DoubleRow k-pairs for MM1
KH2P = KH2 // 2           # 4 DoubleRow k-pairs for MM3
W0_SCALE = 64.0           # W0 pre-scaled into fp8 range; act un-scales
DW1_SCALE = 256.0


def _build(loop_k: int = 1, stages: str = 'ABCD'):
    import contextlib
    import concourse.bacc as bacc
    import concourse.mybir as mybir
    import concourse.tile as tile

    f32 = mybir.dt.float32
    bf16 = mybir.dt.bfloat16
    f8 = mybir.dt.float8e4
    DR = mybir.MatmulPerfMode.DoubleRow
    SIG = mybir.ActivationFunctionType.Sigmoid

    nc = bacc.Bacc("TRN2", target_bir_lowering=False, debug=False)
    XTR = nc.dram_tensor("XTR", (128, KD * BC), f8, kind="ExternalInput").ap()
    W0R = nc.dram_tensor("W0R", (M1, 128, KD * 128), f8,
                         kind="ExternalInput").ap()
    W1R = nc.dram_tensor("W1R", (2, 128, KH1 * 512), bf16,
                         kind="ExternalInput").ap()
    DW1R = nc.dram_tensor("DW1R", (M3, 128, KH2 * 128), f8,
                          kind="ExternalInput").ap()
    DW0R = nc.dram_tensor("DW0R", (M4, 128, KH1 * 128), bf16,
                          kind="ExternalInput").ap()
    B1R = nc.dram_tensor("B1R", (1, H2), bf16, kind="ExternalInput").ap()
    B0R = nc.dram_tensor("B0R", (128, M1), f32, kind="ExternalInput").ap()
    DB1R = nc.dram_tensor("DB1R", (128, M3), f32, kind="ExternalInput").ap()
    DB0R = nc.dram_tensor("DB0R", (128, M4), f32, kind="ExternalInput").ap()
    IDENT = nc.dram_tensor("IDENT", (128, 128), bf16, kind="ExternalInput").ap()
    ONESR = nc.dram_tensor("ONESR", (1, 128), bf16, kind="ExternalInput").ap()
    OUTT = nc.dram_tensor("OUTT", (M4, 128, BC), f32, kind="ExternalOutput").ap()

    with tile.TileContext(nc) as tc:
        loop_cm = tc.For_i(0, loop_k, 1) if loop_k > 1 else contextlib.nullcontext()
        with loop_cm, \
             tc.tile_pool(name="biasp", bufs=1) as biasp, \
             tc.tile_pool(name="cstp", bufs=1) as cstp, \
             tc.tile_pool(name="psum", bufs=6, space="PSUM") as psp, \
             tc.tile_pool(name="tps", bufs=2, space="PSUM") as tpsp:
            # merged per-partition biases: [b0 | db1 | db0] (fp32, act bias)
            biases = biasp.tile([128, M1 + M3 + M4], f32)
            nc.sync.dma_start(biases[:, 0:M1], B0R)
            nc.sync.dma_start(biases[:, M1:M1 + M3], DB1R)
            nc.sync.dma_start(biases[:, M1 + M3:], DB0R)
            b0t = biases[:, 0:M1]
            db1t = biases[:, M1:M1 + M3]
            db0t = biases[:, M1 + M3:M1 + M3 + M4]

            with tc.tile_pool(name="h1T", bufs=1) as h1Tp, \
                 tc.tile_pool(name="w1", bufs=1) as w1p, \
                 tc.tile_pool(name="hmT", bufs=1) as hmTp, \
                 tc.tile_pool(name="h2", bufs=3) as h2p, \
                 tc.tile_pool(name="tk", bufs=2) as tkp, \
                 tc.tile_pool(name="hmp", bufs=5) as hmp, \
                 tc.tile_pool(name="mx8", bufs=2) as mxp:
                # h1T: [128 H1-part x (m-tile, 1024 rows)] bf16, SBUF-resident
                h1T = h1Tp.tile([128, M1 * BC], bf16)
                w1h = w1p.tile([128, 2 * KH1 * 512], bf16)
                # hmaskT, kk-major: [128 H2-part x (kk, 1024 rows)] fp8
                hmTt = hmTp.tile([128, KH2 * BC], f8)
                ident = cstp.tile([128, 128], bf16)
                b1t = cstp.tile([1, H2], bf16)
                ones1 = cstp.tile([1, 128], bf16)

                def mm2_topk(r):
                    """MM2 + sigmoid + top-51 for row tile r (128 rows).
                    PE: 2x(16 mm + rank-1 bias mm); DVE: 7x(max8+match
                    replace); Pool: hmask = h2 - zap (cast to bf16)."""
                    h2r = h2p.tile([128, H2], f32, tag="h2")
                    for n in range(2):
                        ps = psp.tile([128, 512], f32)
                        for kk in range(KH1):
                            nc.tensor.matmul(
                                ps[:],
                                h1T[:, kk * BC + r * 128:
                                    kk * BC + r * 128 + 128],
                                w1h[:, (n * KH1 + kk) * 512:
                                    (n * KH1 + kk) * 512 + 512],
                                start=(kk == 0), stop=False)
                        nc.tensor.matmul(ps[:], ones1[:],
                                         b1t[:, n * 512:(n + 1) * 512],
                                         start=False, stop=True)
                        nc.scalar.activation(h2r[:, n * 512:(n + 1) * 512],
                                             ps[:], SIG)
                    zap = tkp.tile([128, H2], f32, tag="zap")
                    cur = h2r
                    for it in range(7):
                        mx = mxp.tile([128, 8], f32, tag="mx")
                        nc.vector.max(mx[:], cur[:])
                        if it == 6:
                            nc.vector.memset(mx[:, 3:8], 0.0)
                        nc.vector.match_replace(
                            out=zap[:], in_to_replace=mx[:],
                            in_values=cur[:], imm_value=0.0)
                        cur = zap
                    hmask = hmp.tile([128, H2], bf16, tag="hmask")
                    nc.gpsimd.tensor_sub(hmask[:], h2r[:], zap[:])
                    return hmask

                def trans(r, hm):
                    # 8 PE transposes into one psum bank, then a single
                    # strided copy into the kk-major hmTt layout
                    pst = tpsp.tile([128, H2], bf16, name="pst")
                    p3 = pst[:].rearrange("p (kk j) -> p kk j", kk=KH2)
                    for kk in range(KH2):
                        nc.tensor.transpose(
                            p3[:, kk, :], hm[:, kk * 128:(kk + 1) * 128],
                            ident[:])
                    dst = hmTt[:].rearrange(
                        "p (kk bc) -> p kk bc", kk=KH2)[:, :,
                                                        r * 128:(r + 1) * 128]
                    nc.scalar.copy(dst, p3)

                hmasks = {}
                with tc.tile_pool(name="xt", bufs=1) as xtp, \
                     tc.tile_pool(name="w0", bufs=4) as w0p:
                    xt = xtp.tile([128, KD * BC], f8)
                    # rows 0..ROWS_A0 of every k-chunk first (A0's working set)
                    for k in range(KD):
                        nc.scalar.dma_start(xt[:, k * BC:k * BC + ROWS_A0],
                                            XTR[:, k * BC:k * BC + ROWS_A0])
                    nc.scalar.dma_start(ident[:], IDENT)
                    nc.scalar.dma_start(b1t[:], B1R)
                    nc.scalar.dma_start(ones1[:], ONESR)

                    xt4 = xt[:].rearrange("p (j i bc) -> p j i bc",
                                          j=KDP, i=2)

                    def stageA(row_off, nrows, half, hook=None):
                        for m in range(M1):
                            if hook is not None:
                                hook(m)
                            w0s = w0p.tile([128, KD * 128], f8, tag="w0s",
                                           name=f"w0s{half}_{m}")
                            nc.sync.dma_start(w0s[:], W0R[m])
                            w0s4 = w0s[:].rearrange(
                                "p (j i c) -> p j i c", j=KDP, i=2)
                            o = 0
                            while o < nrows:
                                w = min(512, nrows - o)
                                ps = psp.tile([128, 512], f32)
                                c = row_off + o
                                for j in range(KDP):
                                    nc.tensor.matmul(
                                        ps[:, 0:w], w0s4[:, j],
                                        xt4[:, j, :, c:c + w],
                                        start=(j == 0), stop=(j == KDP - 1),
                                        perf_mode=DR)
                                nc.scalar.activation(
                                    h1T[:, m * BC + row_off + o:
                                        m * BC + row_off + o + w],
                                    ps[:, 0:w], SIG, bias=b0t[:, m:m + 1],
                                    scale=1.0 / W0_SCALE)
                                o += w

                    HF = KH1 * 256

                    def prefetch(m):
                        # after A0 m-tile m's act: 3 xt-n1 chunks, then a W1
                        # quarter every 4th m (scalar ring, staggered)
                        for k in range(3 * m, min(3 * m + 3, KD)):
                            nc.scalar.dma_start(
                                xt[:, k * BC + ROWS_A0:k * BC + BC],
                                XTR[:, k * BC + ROWS_A0:k * BC + BC])
                        if m in (4, 7, 10, 13):
                            q = (m - 4) // 3
                            nc.scalar.dma_start(
                                w1h[:, q * HF:(q + 1) * HF],
                                W1R[q // 2, :, (q % 2) * HF:(q % 2 + 1) * HF])

                    if 'A' in stages:
                        stageA(0, ROWS_A0, 0, hook=prefetch)
                    else:
                        for m in range(M1):
                            prefetch(m)
                    if 'B' in stages:
                        for r in range(4):
                            hmasks[r] = mm2_topk(r)
                    if 'A' in stages:
                        stageA(ROWS_A0, ROWS_A1, 1)

                # xt + W0 pools closed: their SBUF is reused below
                with tc.tile_pool(name="dw1", bufs=1) as dw1p, \
                     tc.tile_pool(name="dT", bufs=1) as dTp, \
                     tc.tile_pool(name="dw0", bufs=6) as dw0p, \
                     tc.tile_pool(name="outp", bufs=4) as outp:
                    dw1 = dw1p.tile([128, M3 * KH2 * 128], f8)
                    for m in range(M3):
                        nc.sync.dma_start(
                            dw1[:, m * KH2 * 128:(m + 1) * KH2 * 128],
                            DW1R[m])
                    dT = [dTp.tile([128, BC], bf16, tag=f"dT{m}",
                                   name=f"dT{m}") for m in range(M3)]

                    if 'B' in stages:
                        for r in range(4, 8):
                            hmasks[r] = mm2_topk(r)
                        for r in range(5):
                            trans(r, hmasks.pop(r))

                    hmT4 = hmTt[:].rearrange("p (j i bc) -> p j i bc",
                                             j=KH2P, i=2)
                    dw14 = dw1[:].rearrange("p (m j i c) -> p m j i c",
                                            m=M3, j=KH2P, i=2)

                    def stageC(n2):
                        for m in range(M3):
                            ps = psp.tile([128, 512], f32)
                            for j in range(KH2P):
                                nc.tensor.matmul(
                                    ps[:], dw14[:, m, j],
                                    hmT4[:, j, :, n2 * 512:(n2 + 1) * 512],
                                    start=(j == 0), stop=(j == KH2P - 1),
                                    perf_mode=DR)
                            nc.scalar.activation(
                                dT[m][:, n2 * 512:(n2 + 1) * 512], ps[:],
                                SIG, bias=db1t[:, m:m + 1],
                                scale=1.0 / DW1_SCALE)

                    def stageD(n2):
                        # one 512-row sweep over all 32 out tiles; dw0 is
                        # re-streamed per sweep (DMA has headroom, and this
                        # lets sweep 0 start right after C0 so the last
                        # top-k chains hide under it)
                        for m in range(M4):
                            dw0s = dw0p.tile([128, KH1 * 128], bf16,
                                             tag="dw0s")
                            nc.sync.dma_start(dw0s[:], DW0R[m])
                            om = outp.tile([128, 512], f32, tag="om")
                            ps = psp.tile([128, 512], f32)
                            for kk in range(KH1):
                                nc.tensor.matmul(
                                    ps[:], dw0s[:, kk * 128:(kk + 1) * 128],
                                    dT[kk][:, n2 * 512:(n2 + 1) * 512],
                                    start=(kk == 0), stop=(kk == KH1 - 1))
                            nc.vector.tensor_scalar_add(
                                om[:], ps[:], db0t[:, m:m + 1])
                            nc.scalar.dma_start(
                                OUTT[m][:, n2 * 512:(n2 + 1) * 512], om[:])

                    if 'C' in stages:
                        stageC(0)
                    if 'D' in stages:
                        stageD(0)
                    if 'B' in stages:
                        for r in range(5, 8):
                            trans(r, hmasks.pop(r))
                    if 'C' in stages:
                        stageC(1)
                    if 'D' in stages:
                        stageD(1)
    nc.compile()
    return nc


_NC_CACHE = None


def _get_nc():
    global _NC_CACHE
    if _NC_CACHE is None:
        _NC_CACHE = _build()
    return _NC_CACHE


def _build_looped(loop_k: int):
    return _build(loop_k)


def make_in_maps(x, enc_W0, enc_b0, enc_W1, enc_b1, dec_W1, dec_b1, dec_W0,
                 dec_b0):
    F8 = ml_dtypes.float8_e4m3

    def bf(a):
        return np.asarray(a, np.float32).astype(BF)

    # fp8 DoubleRow pairing: k-chunks (2j, 2j+1) interleave along the free
    # axis as [j, i, .] with i the sub-chunk
    w0r = (np.asarray(enc_W0, np.float32) * W0_SCALE).astype(F8) \
        .reshape(KDP, 2, 128, M1, 128) \
        .transpose(3, 2, 0, 1, 4).reshape(M1, 128, KD * 128)
    w1r = bf(enc_W1).reshape(KH1, 128, 2, 512) \
        .transpose(2, 1, 0, 3).reshape(2, 128, KH1 * 512)
    dw1r = (np.asarray(dec_W1, np.float32) * DW1_SCALE).astype(F8) \
        .reshape(KH2P, 2, 128, M3, 128) \
        .transpose(3, 2, 0, 1, 4).reshape(M3, 128, KH2 * 128)
    dw0r = bf(dec_W0).reshape(KH1, 128, M4, 128) \
        .transpose(2, 1, 0, 3).reshape(M4, 128, KH1 * 128)
    b1r = bf(enc_b1).reshape(1, H2)
    b0r = np.ascontiguousarray(enc_b0.reshape(M1, 128).T, dtype=np.float32)
    db1r = np.ascontiguousarray(dec_b1.reshape(M3, 128).T, dtype=np.float32)
    db0r = np.ascontiguousarray(dec_b0.reshape(M4, 128).T, dtype=np.float32)
    ident = np.eye(128, dtype=np.float32).astype(BF)
    shared = dict(W0R=np.ascontiguousarray(w0r), W1R=np.ascontiguousarray(w1r),
                  DW1R=np.ascontiguousarray(dw1r),
                  DW0R=np.ascontiguousarray(dw0r), B1R=b1r, B0R=b0r,
                  DB1R=db1r, DB0R=db0r, IDENT=ident,
                  ONESR=np.ones((1, 128), dtype=np.float32).astype(BF))
    xr = np.asarray(x, np.float32).astype(F8)
    in_maps = []
    for c in range(NCORES):
        shard = xr[c * BC:(c + 1) * BC]          # [BC, D] fp8
        xt = np.ascontiguousarray(
            shard.T.reshape(KDP, 2, 128, BC).transpose(2, 0, 1, 3)
        ).reshape(128, KD * BC)
        in_maps.append(dict(shared, XTR=xt))
    return in_maps


def kernel(**inputs) -> np.ndarray:
    from concourse import bass_utils
    nc = _get_nc()
    in_maps = make_in_maps(**inputs)
    res = bass_utils.run_bass_kernel_spmd(nc, in_maps,
                                          core_ids=list(range(NCORES)))
    outs = []
    for c in range(NCORES):
        ot = res.results[c]["OUTT"].astype(np.float32)   # [M4, 128, BC]
        outs.append(ot.reshape(D, BC).T)                 # [BC, D]
    return np.ascontiguousarray(np.concatenate(outs, axis=0), dtype=np.float32)


# revision 11
# speedup vs baseline: 1.6044x; 1.0130x over previous
"""Trainium2 Bass kernel for nn_AE_29171417875247 (k-sparse autoencoder with
top-k masking).

  h1 = sigmoid(x @ enc_W0 + enc_b0)        [B, 2048]
  h2 = sigmoid(h1 @ enc_W1 + enc_b1)       [B, 1024]
  h2 = keep top-51 per row, zero rest      (k = 1024 * 0.05)
  d  = sigmoid(h2 @ dec_W1 + dec_b1)       [B, 2048]
  out = d @ dec_W0 + dec_b0                [B, 4096]

Data-parallel across 8 NeuronCores: each core owns 1024 rows of the batch
and the full (replicated) weights. All matmul operands are bf16 (cast on
host, round-to-nearest-even); PSUM accumulation is fp32, and the sigmoid
outputs feeding top-k stay fp32 so the top-51 selection is (near-)exact.
Host-emulated end-to-end bf16 error vs the f32 reference: 4.6e-3.

Per-core pipeline (order chosen so the serial DVE top-k chains hide under
PE-heavy phases; stage A is split 640/384 rows so both top-k batches get a
long PE window):
  A0: h1T rows 0-639   (W0 streamed, xt resident)
  B0-4: MM2+sigmoid rows 0-639 -> 5 DVE top-k chains run during A1
  A1: h1T rows 640-1023 (W0 re-streamed, ~82us of PE to cover the DVE)
  T0-4: PE-transpose hmask tiles 0-4
  B5-7: MM2 rows 640-1023 -> 3 more DVE chains, hidden under C0
  C0: dT[:, 0:512]   = sigmoid(dW1.T @ hmaskT[:, 0:512])
  T5-7, C1: remaining transposes + dT[:, 512:1024]
  D:  outT = dW0.T @ dT + db0 -> DRAM (dw0 streamed once)
"""
import sys
sys.path.insert(0, '/opt/trn_rl_repo')
import numpy as np
import ml_dtypes

BF = ml_dtypes.bfloat16

B, D, H1, H2 = 8192, 4096, 2048, 1024
NCORES = 8
BC = B // NCORES          # rows per core = 1024
K_TOP = 51                # int(H2 * 0.05)
KD = D // 128             # 32 k-chunks for MM1
KH1 = H1 // 128           # 16
KH2 = H2 // 128           # 8
M1 = H1 // 128            # 16 h1 tiles
M3 = H1 // 128            # 16 dT tiles
M4 = D // 128             # 32 out tiles
ROWS_A0 = 512             # stage-A row split: 4 top-k tiles then 4
ROWS_A1 = BC - ROWS_A0    # 512
KDP = KD // 2             # 16 DoubleRow k-pairs for MM1
KH2P = KH2 // 2           # 4 DoubleRow k-pairs for MM3
W0_SCALE = 64.0           # W0 pre-scaled into fp8 range; act un-scales
DW1_SCALE = 256.0


def _build(loop_k: int = 1, stages: str = 'ABCD'):
    import contextlib
    import concourse.bacc as bacc
    import concourse.mybir as mybir
    import concourse.tile as tile

    f32 = mybir.dt.float32
    bf16 = mybir.dt.bfloat16
    f8 = mybir.dt.float8e4
    DR = mybir.MatmulPerfMode.DoubleRow
    SIG = mybir.ActivationFunctionType.Sigmoid

    nc = bacc.Bacc("TRN2", target_bir_lowering=False, debug=False)
    XTR = nc.dram_tensor("XTR", (128, KD * BC), f8, kind="ExternalInput").ap()
    W0R = nc.dram_tensor("W0R", (M1, 128, KD * 128), f8,
                         kind="ExternalInput").ap()
    W1R = nc.dram_tensor("W1R", (2, 128, KH1 * 512), bf16,
                         kind="ExternalInput").ap()
    DW1R = nc.dram_tensor("DW1R", (M3, 128, KH2 * 128), f8,
                          kind="ExternalInput").ap()
    DW0R = nc.dram_tensor("DW0R", (M4, 128, KH1 * 128), bf16,
                          kind="ExternalInput").ap()
    B1R = nc.dram_tensor("B1R", (1, H2), bf16, kind="ExternalInput").ap()
    B0R = nc.dram_tensor("B0R", (128, M1), f32, kind="ExternalInput").ap()
    DB1R = nc.dram_tensor("DB1R", (128, M3), f32, kind="ExternalInput").ap()
    DB0R = nc.dram_tensor("DB0R", (128, M4), f32, kind="ExternalInput").ap()
    IDENT = nc.dram_tensor("IDENT", (128, 128), bf16, kind="ExternalInput").ap()
    ONESR = nc.dram_tensor("ONESR", (1, 128), bf16, kind="ExternalInput").ap()
    OUTT = nc.dram_tensor("OUTT", (M4, 128, BC), f32, kind="ExternalOutput").ap()

    with tile.TileContext(nc) as tc:
        loop_cm = tc.For_i(0, loop_k, 1) if loop_k > 1 else contextlib.nullcontext()
        with loop_cm, \
             tc.tile_pool(name="biasp", bufs=1) as biasp, \
             tc.tile_pool(name="cstp", bufs=1) as cstp, \
             tc.tile_pool(name="psum", bufs=6, space="PSUM") as psp, \
             tc.tile_pool(name="tps", bufs=2, space="PSUM") as tpsp:
            # merged per-partition biases: [b0 | db1 | db0] (fp32, act bias)
            biases = biasp.tile([128, M1 + M3 + M4], f32)
            nc.sync.dma_start(biases[:, 0:M1], B0R)
            nc.sync.dma_start(biases[:, M1:M1 + M3], DB1R)
            nc.sync.dma_start(biases[:, M1 + M3:], DB0R)
            b0t = biases[:, 0:M1]
            db1t = biases[:, M1:M1 + M3]
            db0t = biases[:, M1 + M3:M1 + M3 + M4]

            with tc.tile_pool(name="h1T", bufs=1) as h1Tp, \
                 tc.tile_pool(name="w1", bufs=1) as w1p, \
                 tc.tile_pool(name="hmT", bufs=1) as hmTp, \
                 tc.tile_pool(name="h2", bufs=3) as h2p, \
                 tc.tile_pool(name="tk", bufs=2) as tkp, \
                 tc.tile_pool(name="hmp", bufs=5) as hmp, \
                 tc.tile_pool(name="mx8", bufs=2) as mxp:
                # h1T: [128 H1-part x (m-tile, 1024 rows)] bf16, SBUF-resident
                h1T = h1Tp.tile([128, M1 * BC], bf16)
                w1h = w1p.tile([128, 2 * KH1 * 512], bf16)
                # hmaskT, kk-major: [128 H2-part x (kk, 1024 rows)] fp8
                hmTt = hmTp.tile([128, KH2 * BC], f8)
                ident = cstp.tile([128, 128], bf16)
                b1t = cstp.tile([1, H2], bf16)
                ones1 = cstp.tile([1, 128], bf16)

                def mm2_topk(r):
                    """MM2 + sigmoid + top-51 for row tile r (128 rows).
                    PE: 2x(16 mm + rank-1 bias mm); DVE: 7x(max8+match
                    replace); Pool: hmask = h2 - zap (cast to bf16)."""
                    h2r = h2p.tile([128, H2], f32, tag="h2")
                    for n in range(2):
                        ps = psp.tile([128, 512], f32)
                        for kk in range(KH1):
                            nc.tensor.matmul(
                                ps[:],
                                h1T[:, kk * BC + r * 128:
                                    kk * BC + r * 128 + 128],
                                w1h[:, (n * KH1 + kk) * 512:
                                    (n * KH1 + kk) * 512 + 512],
                                start=(kk == 0), stop=False)
                        nc.tensor.matmul(ps[:], ones1[:],
                                         b1t[:, n * 512:(n + 1) * 512],
                                         start=False, stop=True)
                        nc.scalar.activation(h2r[:, n * 512:(n + 1) * 512],
                                             ps[:], SIG)
                    zap = tkp.tile([128, H2], f32, tag="zap")
                    cur = h2r
                    for it in range(7):
                        mx = mxp.tile([128, 8], f32, tag="mx")
                        nc.vector.max(mx[:], cur[:])
                        if it == 6:
                            nc.vector.memset(mx[:, 3:8], 0.0)
                        nc.vector.match_replace(
                            out=zap[:], in_to_replace=mx[:],
                            in_values=cur[:], imm_value=0.0)
                        cur = zap
                    hmask = hmp.tile([128, H2], bf16, tag="hmask")
                    nc.gpsimd.tensor_sub(hmask[:], h2r[:], zap[:])
                    return hmask

                def trans(r, hm):
                    # 8 PE transposes into one psum bank, then a single
                    # strided copy into the kk-major hmTt layout
                    pst = tpsp.tile([128, H2], bf16, name="pst")
                    p3 = pst[:].rearrange("p (kk j) -> p kk j", kk=KH2)
                    for kk in range(KH2):
                        nc.tensor.transpose(
                            p3[:, kk, :], hm[:, kk * 128:(kk + 1) * 128],
                            ident[:])
                    dst = hmTt[:].rearrange(
                        "p (kk bc) -> p kk bc", kk=KH2)[:, :,
                                                        r * 128:(r + 1) * 128]
                    nc.scalar.copy(dst, p3)

                hmasks = {}
                with tc.tile_pool(name="xt", bufs=1) as xtp, \
                     tc.tile_pool(name="w0", bufs=1) as w0p:
                    xt = xtp.tile([128, KD * BC], f8)
                    w0a = w0p.tile([128, M1 * KD * 128], f8)
                    w0a5 = w0a[:].rearrange(
                        "p (m j i c) -> p m j i c", m=M1, j=KDP, i=2)
                    # rows 0..ROWS_A0 of every k-chunk first (A0's working set)
                    for k in range(KD):
                        nc.scalar.dma_start(xt[:, k * BC:k * BC + ROWS_A0],
                                            XTR[:, k * BC:k * BC + ROWS_A0])
                    nc.scalar.dma_start(ident[:], IDENT)
                    nc.scalar.dma_start(b1t[:], B1R)
                    nc.scalar.dma_start(ones1[:], ONESR)

                    xt4 = xt[:].rearrange("p (j i bc) -> p j i bc",
                                          j=KDP, i=2)

                    def stageA(row_off, nrows, half, hook=None):
                        for m in range(M1):
                            if hook is not None:
                                hook(m)
                            if half == 0:
                                nc.sync.dma_start(
                                    w0a[:, m * KD * 128:(m + 1) * KD * 128],
                                    W0R[m])
                            o = 0
                            while o < nrows:
                                w = min(512, nrows - o)
                                ps = psp.tile([128, 512], f32)
                                c = row_off + o
                                for j in range(KDP):
                                    nc.tensor.matmul(
                                        ps[:, 0:w], w0a5[:, m, j],
                                        xt4[:, j, :, c:c + w],
                                        start=(j == 0), stop=(j == KDP - 1),
                                        perf_mode=DR)
                                nc.scalar.activation(
                                    h1T[:, m * BC + row_off + o:
                                        m * BC + row_off + o + w],
                                    ps[:, 0:w], SIG, bias=b0t[:, m:m + 1],
                                    scale=1.0 / W0_SCALE)
                                o += w

                    HF = KH1 * 256

                    def prefetch(m):
                        # after A0 m-tile m's act: 3 xt-n1 chunks, then a W1
                        # quarter every 4th m (scalar ring, staggered)
                        for k in range(3 * m, min(3 * m + 3, KD)):
                            nc.scalar.dma_start(
                                xt[:, k * BC + ROWS_A0:k * BC + BC],
                                XTR[:, k * BC + ROWS_A0:k * BC + BC])
                        if m in (4, 7, 10, 13):
                            q = (m - 4) // 3
                            nc.scalar.dma_start(
                                w1h[:, q * HF:(q + 1) * HF],
                                W1R[q // 2, :, (q % 2) * HF:(q % 2 + 1) * HF])

                    if 'A' in stages:
                        stageA(0, ROWS_A0, 0, hook=prefetch)
                    else:
                        for m in range(M1):
                            prefetch(m)
                    if 'B' in stages:
                        for r in range(4):
                            hmasks[r] = mm2_topk(r)
                    if 'A' in stages:
                        stageA(ROWS_A0, ROWS_A1, 1)

                # xt + W0 pools closed: their SBUF is reused below
                with tc.tile_pool(name="dw1", bufs=1) as dw1p, \
                     tc.tile_pool(name="dT", bufs=1) as dTp, \
                     tc.tile_pool(name="dw0", bufs=6) as dw0p, \
                     tc.tile_pool(name="outp", bufs=4) as outp:
                    dw1 = dw1p.tile([128, M3 * KH2 * 128], f8)
                    for m in range(M3):
                        nc.sync.dma_start(
                            dw1[:, m * KH2 * 128:(m + 1) * KH2 * 128],
                            DW1R[m])
                    dT = [dTp.tile([128, BC], bf16, tag=f"dT{m}",
                                   name=f"dT{m}") for m in range(M3)]

                    if 'B' in stages:
                        for r in range(4, 8):
                            hmasks[r] = mm2_topk(r)
                        for r in range(5):
                            trans(r, hmasks.pop(r))

                    hmT4 = hmTt[:].rearrange("p (j i bc) -> p j i bc",
                                             j=KH2P, i=2)
                    dw14 = dw1[:].rearrange("p (m j i c) -> p m j i c",
                                            m=M3, j=KH2P, i=2)

                    def stageC(n2):
                        for m in range(M3):
                            ps = psp.tile([128, 512], f32)
                            for j in range(KH2P):
                                nc.tensor.matmul(
                                    ps[:], dw14[:, m, j],
                                    hmT4[:, j, :, n2 * 512:(n2 + 1) * 512],
                                    start=(j == 0), stop=(j == KH2P - 1),
                                    perf_mode=DR)
                            nc.scalar.activation(
                                dT[m][:, n2 * 512:(n2 + 1) * 512], ps[:],
                                SIG, bias=db1t[:, m:m + 1],
                                scale=1.0 / DW1_SCALE)

                    def stageD(n2):
                        # one 512-row sweep over all 32 out tiles; dw0 is
                        # re-streamed per sweep (DMA has headroom, and this
                        # lets sweep 0 start right after C0 so the last
                        # top-k chains hide under it)
                        for m in range(M4):
                            dw0s = dw0p.tile([128, KH1 * 128], bf16,
                                             tag="dw0s")
                            nc.sync.dma_start(dw0s[:], DW0R[m])
                            om = outp.tile([128, 512], f32, tag="om")
                            ps = psp.tile([128, 512], f32)
                            for kk in range(KH1):
                                nc.tensor.matmul(
                                    ps[:], dw0s[:, kk * 128:(kk + 1) * 128],
                                    dT[kk][:, n2 * 512:(n2 + 1) * 512],
                                    start=(kk == 0), stop=(kk == KH1 - 1))
                            nc.vector.tensor_scalar_add(
                                om[:], ps[:], db0t[:, m:m + 1])
                            nc.scalar.dma_start(
                                OUTT[m][:, n2 * 512:(n2 + 1) * 512], om[:])

                    if 'C' in stages:
                        stageC(0)
                    if 'D' in stages:
                        stageD(0)
                    if 'B' in stages:
                        for r in range(5, 8):
                            trans(r, hmasks.pop(r))
                    if 'C' in stages:
                        stageC(1)
                    if 'D' in stages:
                        stageD(1)
    nc.compile()
    return nc


_NC_CACHE = None


def _get_nc():
    global _NC_CACHE
    if _NC_CACHE is None:
        _NC_CACHE = _build()
    return _NC_CACHE


def _build_looped(loop_k: int):
    return _build(loop_k)


def make_in_maps(x, enc_W0, enc_b0, enc_W1, enc_b1, dec_W1, dec_b1, dec_W0,
                 dec_b0):
    F8 = ml_dtypes.float8_e4m3

    def bf(a):
        return np.asarray(a, np.float32).astype(BF)

    # fp8 DoubleRow pairing: k-chunks (2j, 2j+1) interleave along the free
    # axis as [j, i, .] with i the sub-chunk
    w0r = (np.asarray(enc_W0, np.float32) * W0_SCALE).astype(F8) \
        .reshape(KDP, 2, 128, M1, 128) \
        .transpose(3, 2, 0, 1, 4).reshape(M1, 128, KD * 128)
    w1r = bf(enc_W1).reshape(KH1, 128, 2, 512) \
        .transpose(2, 1, 0, 3).reshape(2, 128, KH1 * 512)
    dw1r = (np.asarray(dec_W1, np.float32) * DW1_SCALE).astype(F8) \
        .reshape(KH2P, 2, 128, M3, 128) \
        .transpose(3, 2, 0, 1, 4).reshape(M3, 128, KH2 * 128)
    dw0r = bf(dec_W0).reshape(KH1, 128, M4, 128) \
        .transpose(2, 1, 0, 3).reshape(M4, 128, KH1 * 128)
    b1r = bf(enc_b1).reshape(1, H2)
    b0r = np.ascontiguousarray(enc_b0.reshape(M1, 128).T, dtype=np.float32)
    db1r = np.ascontiguousarray(dec_b1.reshape(M3, 128).T, dtype=np.float32)
    db0r = np.ascontiguousarray(dec_b0.reshape(M4, 128).T, dtype=np.float32)
    ident = np.eye(128, dtype=np.float32).astype(BF)
    shared = dict(W0R=np.ascontiguousarray(w0r), W1R=np.ascontiguousarray(w1r),
                  DW1R=np.ascontiguousarray(dw1r),
                  DW0R=np.ascontiguousarray(dw0r), B1R=b1r, B0R=b0r,
                  DB1R=db1r, DB0R=db0r, IDENT=ident,
                  ONESR=np.ones((1, 128), dtype=np.float32).astype(BF))
    xr = np.asarray(x, np.float32).astype(F8)
    in_maps = []
    for c in range(NCORES):
        shard = xr[c * BC:(c + 1) * BC]          # [BC, D] fp8
        xt = np.ascontiguousarray(
            shard.T.reshape(KDP, 2, 128, BC).transpose(2, 0, 1, 3)
        ).reshape(128, KD * BC)
        in_maps.append(dict(shared, XTR=xt))
    return in_maps


def kernel(**inputs) -> np.ndarray:
    from concourse import bass_utils
    nc = _get_nc()
    in_maps = make_in_maps(**inputs)
    res = bass_utils.run_bass_kernel_spmd(nc, in_maps,
                                          core_ids=list(range(NCORES)))
    outs = []
    for c in range(NCORES):
        ot = res.results[c]["OUTT"].astype(np.float32)   # [M4, 128, BC]
        outs.append(ot.reshape(D, BC).T)                 # [BC, D]
    return np.ascontiguousarray(np.concatenate(outs, axis=0), dtype=np.float32)
